# revision 25
# baseline (speedup 1.0000x reference)
"""GQA attention (B=2, S=2048, DM=1024, H=16, KH=4, RoPE, causal) on 8 TRN2 cores.

Sharding: DP=2 over batch x TP=4 over heads. Core c handles batch c//4 and
q-heads [4r, 4r+4), kv-head r, where r = c % 4. Each core computes a partial
out^T = wo_shard @ attn_shard of shape [DM, S]; the host sums the 4 partials
per batch and transposes (gather/unshard).

Per-core kernel (single NEFF, SPMD):
  - Q/K computed feature-major ([feat, tok]) via transposed weight layouts
    prepared on host; V computed feature-major then PE-transposed to
    token-major with a ones-column appended (rowsum trick).
  - RoPE: adjacent-partition swap via stream_shuffle + elementwise muls with
    replicated cos / (+-)sin tables.
  - Causal softmax without max-subtraction (logits are bounded ~|2.5| for
    this distribution); exp on ACT engine over two PSUM banks at once
    (both heads of a pair); causal masking as a post-exp bf16 multiply on
    the 4 diagonal key-blocks only.
  - Scores matmuls run two heads concurrently in the two 64-row PE groups
    (K duplicated to partitions 64..127).
  - All dense matmuls in float32r (full PE rate at free-dim 512); AV in bf16.
"""

import numpy as np
import ml_dtypes

import concourse.bass as bass
import concourse.mybir as mybir
import concourse.tile as tile
from concourse import bacc
from concourse.bass_utils import run_bass_kernel_spmd
from concourse.masks import make_identity

F32 = mybir.dt.float32
F32R = mybir.dt.float32r
BF16 = mybir.dt.bfloat16

B, S, DM, H, KH, HD = 2, 2048, 1024, 16, 4, 64
N_CORES = 8
TPG = 4                 # tensor-parallel group size
QH = H // TPG           # q-heads per core
KFEAT = QH * HD         # 256 q-features per core
SC = 512                # token chunk
NCH = S // SC           # 4
KB = 128                # key block
NKB = S // KB           # 16
SCALE = 1.0 / np.sqrt(HD)
XOR1 = [i ^ 1 for i in range(32)]

LAST_RESULTS = None     # BassKernelResults of the most recent run (for test.py)
_NC_CACHE = None


def build_nc():
    nc = bacc.Bacc("TRN2", target_bir_lowering=False, debug=False,
                   num_devices=N_CORES)

    xT = nc.declare_dram_parameter("xT", [DM, S], BF16, isOutput=False)
    wqT = nc.declare_dram_parameter("wqT", [DM, KFEAT], BF16, isOutput=False)
    wkvT = nc.declare_dram_parameter("wkvT", [DM, 128], BF16, isOutput=False)
    woT = nc.declare_dram_parameter("woT", [KFEAT, DM], BF16, isOutput=False)
    ropeCos = nc.declare_dram_parameter("ropeCos", [64, S], F32, isOutput=False)
    ropeSin = nc.declare_dram_parameter("ropeSin", [64, S], F32, isOutput=False)
    mask01 = nc.declare_dram_parameter("mask01", [128, 896], BF16, isOutput=False)
    out = nc.declare_dram_parameter("out", [DM, S], F32, isOutput=True)

    xT_v = xT.rearrange("(kb p) n -> p kb n", p=128)        # [128, 8, S]
    wqT_v = wqT.rearrange("(kb p) m -> p kb m", p=128)      # [128, 8, 256]
    wkvT_v = wkvT.rearrange("(kb p) m -> p kb m", p=128)    # [128, 8, 128]
    woT_v = woT.rearrange("(c p) n -> p c n", p=128)        # [128, 2, 1024]
    out_v = out.rearrange("(mb p) n -> p mb n", p=128)      # [128, 8, S]

    EXP = mybir.ActivationFunctionType.Exp
    MUL = bass.mybir.AluOpType.mult
    ADD = bass.mybir.AluOpType.add

    with tile.TileContext(nc) as tc:
        with (
            tc.tile_pool(name="consts", bufs=1) as consts,
            tc.tile_pool(name="kch", bufs=NCH) as kch_pool,
            tc.tile_pool(name="qch", bufs=NCH) as qch_pool,
            tc.tile_pool(name="ach", bufs=NCH) as ach_pool,
            tc.tile_pool(name="v1p", bufs=NKB) as v1_pool,
            tc.tile_pool(name="xch", bufs=2) as xch_pool,
            tc.tile_pool(name="tmp", bufs=2) as tmp_pool,
            tc.tile_pool(name="pp", bufs=5) as p_pool,
            tc.tile_pool(name="rp", bufs=2) as r_pool,
            tc.tile_pool(name="op", bufs=3) as o_pool,
            tc.tile_pool(name="ocp", bufs=6) as oc_pool,
            tc.tile_pool(name="acc", bufs=2, space="PSUM") as acc_pool,
            tc.tile_pool(name="oac", bufs=1, space="PSUM") as oacc_pool,
            tc.tile_pool(name="sme", bufs=2, space="PSUM") as s_pool,
        ):
            # ---- constants ----
            wq_sb = consts.tile([128, 8, KFEAT], BF16, tag="wq")
            wkv_sb = consts.tile([128, 8, 128], BF16, tag="wkv")
            wo_sb = consts.tile([128, 2, DM], BF16, tag="wo")
            cos_sb = consts.tile([128, S], F32, tag="cos")
            sin_sb = consts.tile([128, S], F32, tag="sin")
            mask_sb = consts.tile([128, 896], BF16, tag="mask")
            ident = consts.tile([128, 128], BF16, tag="ident")

            nc.sync.dma_start(wq_sb[:], wqT_v)
            nc.sync.dma_start(wkv_sb[:], wkvT_v)
            nc.sync.dma_start(wo_sb[:], woT_v)
            nc.sync.dma_start(cos_sb[0:64, :], ropeCos[:])
            nc.sync.dma_start(cos_sb[64:128, :], ropeCos[:])
            nc.sync.dma_start(sin_sb[0:64, :], ropeSin[:])
            nc.sync.dma_start(sin_sb[64:128, :], ropeSin[:])
            nc.sync.dma_start(mask_sb[:], mask01[:])
            make_identity(nc, ident[:])
            ones1f = consts.tile([1, 64], F32, tag="ones1f")
            ones1 = consts.tile([1, 64], F32R, tag="ones1")
            nc.vector.memset(ones1f[:], 1.0)
            nc.vector.tensor_copy(ones1[:], ones1f[:])

            K_ch = []       # per-chunk K, feature-major, duplicated rows
            Q_ch = []       # per-chunk Q, feature-major, [128, 2, SC]
            A_ch = []       # per-chunk attn output, feature-major
            V1_kb = []      # per key-block token-major [V | 1]

            def proj(c0):
                cols = slice(c0 * SC, (c0 + 1) * SC)
                x_sb = xch_pool.tile([128, 8, SC], BF16, tag="x")
                nc.sync.dma_start(x_sb[:], xT_v[:, :, cols])

                q_sb = qch_pool.tile([128, 2, SC], BF16, tag="q")
                k_sb = kch_pool.tile([128, SC], BF16, tag="k")
                Q_ch.append(q_sb)
                K_ch.append(k_sb)

                # Q projection + RoPE, two 128-feature tiles (2 heads each)
                for m in range(2):
                    q_ps = acc_pool.tile([128, SC], F32, tag="acc")
                    for kb in range(8):
                        nc.tensor.matmul(
                            q_ps[:],
                            wq_sb[:, kb, m * 128:(m + 1) * 128],
                            x_sb[:, kb, :],
                            start=(kb == 0), stop=(kb == 7),
                        )
                    qsw = tmp_pool.tile([128, SC], F32, tag="qsw")
                    t1 = tmp_pool.tile([128, SC], F32, tag="t1")
                    t2 = tmp_pool.tile([128, SC], F32, tag="t2")
                    nc.vector.stream_shuffle(qsw[:], q_ps[:], XOR1)
                    nc.vector.tensor_tensor(t1[:], q_ps[:], cos_sb[:, cols], MUL)
                    nc.vector.tensor_tensor(t2[:], qsw[:], sin_sb[:, cols], MUL)
                    nc.vector.tensor_tensor(q_sb[:, m, :], t1[:], t2[:], ADD)

                # K (rows 0:64) and V (rows 64:128) projection
                kv_ps = acc_pool.tile([128, SC], F32, tag="acc")
                for kb in range(8):
                    nc.tensor.matmul(
                        kv_ps[:],
                        wkv_sb[:, kb, :],
                        x_sb[:, kb, :],
                        start=(kb == 0), stop=(kb == 7),
                    )
                # K RoPE
                ksw = tmp_pool.tile([64, SC], F32, tag="ksw")
                t1k = tmp_pool.tile([64, SC], F32, tag="t1k")
                t2k = tmp_pool.tile([64, SC], F32, tag="t2k")
                nc.vector.stream_shuffle(ksw[:], kv_ps[0:64, :], XOR1)
                nc.vector.tensor_tensor(t1k[:], kv_ps[0:64, :], cos_sb[0:64, cols], MUL)
                nc.vector.tensor_tensor(t2k[:], ksw[:], sin_sb[0:64, cols], MUL)
                nc.vector.tensor_tensor(k_sb[0:64, :], t1k[:], t2k[:], ADD)
                # duplicate K into partitions 64:128 (second PE row group)
                nc.sync.dma_start(k_sb[64:128, :], k_sb[0:64, :])

                # V: cast to bf16, PE-transpose to token-major, append ones col
                vtmp = tmp_pool.tile([128, SC], BF16, tag="vtmp")
                nc.vector.tensor_copy(vtmp[64:128, :], kv_ps[64:128, :])
                for tb in range(4):
                    v1 = v1_pool.tile([128, 66], BF16, tag="v1")
                    V1_kb.append(v1)
                    vt_ps = acc_pool.tile([128, 64], BF16, tag="acc")
                    nc.tensor.transpose(
                        vt_ps[:], vtmp[64:128, tb * 128:(tb + 1) * 128],
                        ident[64:128, 64:128],
                    )
                    nc.vector.tensor_copy(v1[:, 0:64], vt_ps[:])
                    nc.vector.memset(v1[:, 64:65], 1.0)

            def attention(c0):
                nkb = 4 * (c0 + 1)
                a_pair = [ach_pool.tile([128, SC], BF16, tag="a",
                                        name=f"a_c{c0}p{i}")
                          for i in range(2)]
                A_ch.append(a_pair)
                for p in range(2):
                    op2 = oacc_pool.tile([65, 2, SC], F32, tag="oacc")
                    o0 = op2[:, 0, :]
                    o1 = op2[:, 1, :]
                    for kb in range(nkb):
                        kc = K_ch[kb // 4]
                        kcols = slice((kb % 4) * 128, (kb % 4 + 1) * 128)
                        s2 = s_pool.tile([128, 2, SC], F32, tag="s2")
                        nc.tensor.matmul(
                            s2[:, 0, :],
                            kc[0:64, kcols],
                            Q_ch[c0][0:64, p, :],
                            start=True, stop=True,
                        )
                        nc.tensor.matmul(
                            s2[:, 1, :],
                            kc[64:128, kcols],
                            Q_ch[c0][64:128, p, :],
                            start=True, stop=True,
                            tile_position=(64, 0),
                        )
                        # on diagonal blocks, only q-cols >= q0 are live
                        j = kb - (nkb - 4)
                        q0 = 128 * j if j >= 0 else 0
                        w = SC - q0
                        p2 = p_pool.tile([128, 2, SC], BF16, tag="p2")
                        nc.scalar.activation(p2[:, :, q0:], s2[:, :, q0:],
                                             EXP, scale=SCALE)
                        if j >= 0:
                            msk = mask_sb[:, 384: 896 - q0]
                            nc.vector.tensor_tensor(
                                p2[:, :, q0:], p2[:, :, q0:],
                                msk.unsqueeze(1).to_broadcast([128, 2, w]), MUL)
                        nc.tensor.matmul(
                            o0[:, q0:], V1_kb[kb][:, 0:65], p2[:, 0, q0:],
                            start=(kb == 0), stop=(kb == nkb - 1),
                        )
                        nc.tensor.matmul(
                            o1[:, q0:], V1_kb[kb][:, 0:65], p2[:, 1, q0:],
                            start=(kb == 0), stop=(kb == nkb - 1),
                        )
                    # evacuate PSUM accumulator to SBUF (frees the banks)
                    oc2 = oc_pool.tile([65, 2, SC], F32, tag="oc")
                    nc.vector.tensor_copy(oc2[:], op2[:])
                    divide_pair(a_pair[p], oc2)
                return a_pair

            def divide_pair(a_tile, oc2):
                # reshape each [1, 512] sums row to [32, 16] so the
                # reciprocal runs on many DVE lanes instead of one
                rsum = r_pool.tile([64, 16], F32, tag="rsum")
                for hh in range(2):
                    nc.sync.dma_start(
                        rsum[32 * hh: 32 * hh + 32, :],
                        oc2[64:65, hh, :].rearrange("o (a n) -> o a n", a=32))
                rrecs = r_pool.tile([64, 16], F32R, tag="rrecs")
                with nc.allow_low_precision(reason="f32r-typed reciprocal output"):
                    nc.vector.reciprocal(rrecs[:], rsum[:])
                rrec = r_pool.tile([1, 2, SC], F32R, tag="rrec")
                for hh in range(2):
                    nc.sync.dma_start(
                        rrec[0:1, hh, :].rearrange("o (a n) -> o a n", a=32),
                        rrecs[32 * hh: 32 * hh + 32, :])
                for hh in range(2):
                    bc = acc_pool.tile([64, SC], F32, tag="acc")
                    nc.tensor.matmul(
                        bc[:], ones1[0:1, :], rrec[0:1, hh, :],
                        start=True, stop=True,
                    )
                    if hh == 0:
                        nc.vector.tensor_tensor(
                            a_tile[0:64, :], oc2[0:64, hh, :], bc[:], MUL)
                    else:
                        tb = r_pool.tile([64, SC], BF16, tag="tb")
                        nc.vector.tensor_tensor(
                            tb[:], oc2[0:64, hh, :], bc[:], MUL)
                        # move to partitions 64:128 (DMA crosses partitions)
                        nc.sync.dma_start(a_tile[64:128, :], tb[:])

            def out_proj(c0):
                ncols = slice(c0 * SC, (c0 + 1) * SC)
                for mb in range(8):
                    o_ps = acc_pool.tile([128, SC], F32, tag="acc")
                    for c in range(2):
                        nc.tensor.matmul(
                            o_ps[:],
                            wo_sb[:, c, mb * 128:(mb + 1) * 128],
                            A_ch[c0][c][:, :],
                            start=(c == 0), stop=(c == 1),
                        )
                    osb = o_pool.tile([128, SC], F32, tag="osb")
                    nc.scalar.copy(osb[:], o_ps[:])
                    nc.sync.dma_start(out_v[:, mb, ncols], osb[:])

            proj(0)
            for c0 in range(NCH):
                attention(c0)
                if c0 + 1 < NCH:
                    proj(c0 + 1)       # fills the PE while division runs
                out_proj(c0)

    nc.compile()
    return nc


def shard_inputs(x, wq, wk, wv, wo, freqs_cos, freqs_sin):
    """Build the 8 per-core input maps (host-side layout prep)."""
    x = np.ascontiguousarray(np.asarray(x, dtype=np.float32))
    wq = np.asarray(wq, dtype=np.float32)
    wk = np.asarray(wk, dtype=np.float32)
    wv = np.asarray(wv, dtype=np.float32)
    wo = np.asarray(wo, dtype=np.float32)
    cos = np.asarray(freqs_cos, dtype=np.float32)   # [S, 32]
    sin = np.asarray(freqs_sin, dtype=np.float32)

    rope_cos = np.repeat(cos.T, 2, axis=0)          # [64, S]
    rope_sin = np.repeat(sin.T, 2, axis=0)
    rope_sin[0::2, :] *= -1.0                       # row 2i: -sin_i, 2i+1: +sin_i

    # causal keep-mask M[k, c] = 1 iff k <= c - 384; tiles slice cols
    kk = np.arange(128)[:, None]
    cc = np.arange(896)[None, :]
    mask01 = (kk <= cc - 384).astype(ml_dtypes.bfloat16)

    in_maps = []
    for core in range(N_CORES):
        b, r = divmod(core, TPG)
        xT = np.ascontiguousarray(x[b].T)                         # [DM, S]
        wq_s = wq[r * KFEAT:(r + 1) * KFEAT]                      # [256, DM]
        wk_s = wk[r * HD:(r + 1) * HD]                            # [64, DM]
        wv_s = wv[r * HD:(r + 1) * HD]
        wkvT = np.ascontiguousarray(
            np.concatenate([wk_s, wv_s], axis=0).T)               # [DM, 128]
        wqT = np.ascontiguousarray(wq_s.T)                        # [DM, 256]
        woT = np.ascontiguousarray(wo[:, r * KFEAT:(r + 1) * KFEAT].T)  # [256, DM]
        bf = ml_dtypes.bfloat16
        in_maps.append({
            "xT": xT.astype(bf),
            "wqT": wqT.astype(bf),
            "wkvT": wkvT.astype(bf),
            "woT": woT.astype(bf),
            "ropeCos": rope_cos,
            "ropeSin": rope_sin,
            "mask01": mask01,
        })
    return in_maps


def unshard(results):
    """Sum TP partials per batch and transpose back to [B, S, DM]."""
    out = np.empty((B, S, DM), dtype=np.float32)
    for b in range(B):
        acc = results[b * TPG]["out"].astype(np.float32).copy()
        for r in range(1, TPG):
            acc += results[b * TPG + r]["out"]
        out[b] = acc.T
    return out


def kernel(**inputs):
    global LAST_RESULTS, _NC_CACHE
    if _NC_CACHE is None:
        _NC_CACHE = build_nc()
    in_maps = shard_inputs(**inputs)
    LAST_RESULTS = run_bass_kernel_spmd(_NC_CACHE, in_maps, list(range(N_CORES)))
    return unshard(LAST_RESULTS.results)


# revision 26
# speedup vs baseline: 1.0138x; 1.0138x over previous
"""GQA attention (B=2, S=2048, DM=1024, H=16, KH=4, RoPE, causal) on 8 TRN2 cores.

Sharding: DP=2 over batch x TP=4 over heads. Core c handles batch c//4 and
q-heads [4r, 4r+4), kv-head r, where r = c % 4. Each core computes a partial
out^T = wo_shard @ attn_shard of shape [DM, S]; the host sums the 4 partials
per batch and transposes (gather/unshard).

Per-core kernel (single NEFF, SPMD):
  - Q/K computed feature-major ([feat, tok]) via transposed weight layouts
    prepared on host; V computed feature-major then PE-transposed to
    token-major with a ones-column appended (rowsum trick).
  - RoPE: adjacent-partition swap via stream_shuffle + elementwise muls with
    replicated cos / (+-)sin tables.
  - Causal softmax without max-subtraction (logits are bounded ~|2.5| for
    this distribution); exp on ACT engine over two PSUM banks at once
    (both heads of a pair); causal masking as a post-exp bf16 multiply on
    the 4 diagonal key-blocks only.
  - Scores matmuls run two heads concurrently in the two 64-row PE groups
    (K duplicated to partitions 64..127).
  - All dense matmuls in float32r (full PE rate at free-dim 512); AV in bf16.
"""

import numpy as np
import ml_dtypes

import concourse.bass as bass
import concourse.mybir as mybir
import concourse.tile as tile
from concourse import bacc
from concourse.bass_utils import run_bass_kernel_spmd
from concourse.masks import make_identity

F32 = mybir.dt.float32
F32R = mybir.dt.float32r
BF16 = mybir.dt.bfloat16

B, S, DM, H, KH, HD = 2, 2048, 1024, 16, 4, 64
N_CORES = 8
TPG = 4                 # tensor-parallel group size
QH = H // TPG           # q-heads per core
KFEAT = QH * HD         # 256 q-features per core
SC = 512                # token chunk
NCH = S // SC           # 4
KB = 128                # key block
NKB = S // KB           # 16
SCALE = 1.0 / np.sqrt(HD)
XOR1 = [i ^ 1 for i in range(32)]

LAST_RESULTS = None     # BassKernelResults of the most recent run (for test.py)
_NC_CACHE = None


def build_nc():
    nc = bacc.Bacc("TRN2", target_bir_lowering=False, debug=False,
                   num_devices=N_CORES)

    xT = nc.declare_dram_parameter("xT", [DM, S], BF16, isOutput=False)
    wqT = nc.declare_dram_parameter("wqT", [DM, KFEAT], BF16, isOutput=False)
    wkvT = nc.declare_dram_parameter("wkvT", [DM, 128], BF16, isOutput=False)
    woT = nc.declare_dram_parameter("woT", [KFEAT, DM], BF16, isOutput=False)
    ropeCos = nc.declare_dram_parameter("ropeCos", [64, S], F32, isOutput=False)
    ropeSin = nc.declare_dram_parameter("ropeSin", [64, S], F32, isOutput=False)
    mask01 = nc.declare_dram_parameter("mask01", [128, 896], BF16, isOutput=False)
    out = nc.declare_dram_parameter("out", [DM, S], F32, isOutput=True)

    xT_v = xT.rearrange("(kb p) n -> p kb n", p=128)        # [128, 8, S]
    wqT_v = wqT.rearrange("(kb p) m -> p kb m", p=128)      # [128, 8, 256]
    wkvT_v = wkvT.rearrange("(kb p) m -> p kb m", p=128)    # [128, 8, 128]
    woT_v = woT.rearrange("(c p) n -> p c n", p=128)        # [128, 2, 1024]
    out_v = out.rearrange("(mb p) n -> p mb n", p=128)      # [128, 8, S]

    EXP = mybir.ActivationFunctionType.Exp
    MUL = bass.mybir.AluOpType.mult
    ADD = bass.mybir.AluOpType.add

    with tile.TileContext(nc) as tc:
        with (
            tc.tile_pool(name="consts", bufs=1) as consts,
            tc.tile_pool(name="kch", bufs=NCH) as kch_pool,
            tc.tile_pool(name="qch", bufs=NCH) as qch_pool,
            tc.tile_pool(name="ach", bufs=NCH) as ach_pool,
            tc.tile_pool(name="v1p", bufs=NKB) as v1_pool,
            tc.tile_pool(name="xch", bufs=2) as xch_pool,
            tc.tile_pool(name="tmp", bufs=2) as tmp_pool,
            tc.tile_pool(name="pp", bufs=5) as p_pool,
            tc.tile_pool(name="rp", bufs=2) as r_pool,
            tc.tile_pool(name="op", bufs=3) as o_pool,
            tc.tile_pool(name="ocp", bufs=6) as oc_pool,
            tc.tile_pool(name="acc", bufs=2, space="PSUM") as acc_pool,
            tc.tile_pool(name="oac", bufs=1, space="PSUM") as oacc_pool,
            tc.tile_pool(name="sme", bufs=2, space="PSUM") as s_pool,
        ):
            # ---- constants ----
            wq_sb = consts.tile([128, 8, KFEAT], BF16, tag="wq")
            wkv_sb = consts.tile([128, 8, 128], BF16, tag="wkv")
            wo_sb = consts.tile([128, 2, DM], BF16, tag="wo")
            cos_sb = consts.tile([128, S], F32, tag="cos")
            sin_sb = consts.tile([128, S], F32, tag="sin")
            mask_sb = consts.tile([128, 896], BF16, tag="mask")
            ident = consts.tile([128, 128], BF16, tag="ident")

            nc.sync.dma_start(wq_sb[:], wqT_v)
            nc.sync.dma_start(wkv_sb[:], wkvT_v)
            nc.sync.dma_start(wo_sb[:], woT_v)
            nc.sync.dma_start(cos_sb[0:64, :], ropeCos[:])
            nc.sync.dma_start(cos_sb[64:128, :], ropeCos[:])
            nc.sync.dma_start(sin_sb[0:64, :], ropeSin[:])
            nc.sync.dma_start(sin_sb[64:128, :], ropeSin[:])
            nc.sync.dma_start(mask_sb[:], mask01[:])
            make_identity(nc, ident[:])
            ones1f = consts.tile([1, 64], F32, tag="ones1f")
            ones1 = consts.tile([1, 64], F32R, tag="ones1")
            nc.vector.memset(ones1f[:], 1.0)
            nc.vector.tensor_copy(ones1[:], ones1f[:])

            K_ch = []       # per-chunk K, feature-major, duplicated rows
            Q_ch = []       # per-chunk Q, feature-major, [128, 2, SC]
            A_ch = []       # per-chunk attn output, feature-major
            V1_kb = []      # per key-block token-major [V | 1]

            def proj(c0):
                cols = slice(c0 * SC, (c0 + 1) * SC)
                x_sb = xch_pool.tile([128, 8, SC], BF16, tag="x")
                nc.sync.dma_start(x_sb[:], xT_v[:, :, cols])

                q_sb = qch_pool.tile([128, 2, SC], BF16, tag="q")
                k_sb = kch_pool.tile([128, SC], BF16, tag="k")
                Q_ch.append(q_sb)
                K_ch.append(k_sb)

                # Q projection + RoPE, two 128-feature tiles (2 heads each)
                for m in range(2):
                    q_ps = acc_pool.tile([128, SC], F32, tag="acc")
                    for kb in range(8):
                        nc.tensor.matmul(
                            q_ps[:],
                            wq_sb[:, kb, m * 128:(m + 1) * 128],
                            x_sb[:, kb, :],
                            start=(kb == 0), stop=(kb == 7),
                        )
                    qsw = tmp_pool.tile([128, SC], F32, tag="qsw")
                    t1 = tmp_pool.tile([128, SC], F32, tag="t1")
                    t2 = tmp_pool.tile([128, SC], F32, tag="t2")
                    nc.vector.stream_shuffle(qsw[:], q_ps[:], XOR1)
                    nc.vector.tensor_tensor(t1[:], q_ps[:], cos_sb[:, cols], MUL)
                    nc.vector.tensor_tensor(t2[:], qsw[:], sin_sb[:, cols], MUL)
                    nc.vector.tensor_tensor(q_sb[:, m, :], t1[:], t2[:], ADD)

                # K (rows 0:64) and V (rows 64:128) projection
                kv_ps = acc_pool.tile([128, SC], F32, tag="acc")
                for kb in range(8):
                    nc.tensor.matmul(
                        kv_ps[:],
                        wkv_sb[:, kb, :],
                        x_sb[:, kb, :],
                        start=(kb == 0), stop=(kb == 7),
                    )
                # K RoPE
                ksw = tmp_pool.tile([64, SC], F32, tag="ksw")
                t1k = tmp_pool.tile([64, SC], F32, tag="t1k")
                t2k = tmp_pool.tile([64, SC], F32, tag="t2k")
                nc.vector.stream_shuffle(ksw[:], kv_ps[0:64, :], XOR1)
                nc.vector.tensor_tensor(t1k[:], kv_ps[0:64, :], cos_sb[0:64, cols], MUL)
                nc.vector.tensor_tensor(t2k[:], ksw[:], sin_sb[0:64, cols], MUL)
                nc.vector.tensor_tensor(k_sb[0:64, :], t1k[:], t2k[:], ADD)
                # duplicate K into partitions 64:128 (second PE row group)
                nc.sync.dma_start(k_sb[64:128, :], k_sb[0:64, :])

                # V: cast to bf16, PE-transpose to token-major, append ones col
                vtmp = tmp_pool.tile([128, SC], BF16, tag="vtmp")
                nc.vector.tensor_copy(vtmp[64:128, :], kv_ps[64:128, :])
                for tb in range(4):
                    v1 = v1_pool.tile([128, 66], BF16, tag="v1")
                    V1_kb.append(v1)
                    vt_ps = acc_pool.tile([128, 64], BF16, tag="acc")
                    nc.tensor.transpose(
                        vt_ps[:], vtmp[64:128, tb * 128:(tb + 1) * 128],
                        ident[64:128, 64:128],
                    )
                    nc.vector.tensor_copy(v1[:, 0:64], vt_ps[:])
                    nc.vector.memset(v1[:, 64:65], 1.0)

            def attention(c0):
                nkb = 4 * (c0 + 1)
                a_pair = [ach_pool.tile([128, SC], BF16, tag="a",
                                        name=f"a_c{c0}p{i}")
                          for i in range(2)]
                A_ch.append(a_pair)
                for p in range(2):
                    op2 = oacc_pool.tile([65, 2, SC], F32, tag="oacc")
                    o0 = op2[:, 0, :]
                    o1 = op2[:, 1, :]
                    for kb in range(nkb):
                        kc = K_ch[kb // 4]
                        kcols = slice((kb % 4) * 128, (kb % 4 + 1) * 128)
                        s2 = s_pool.tile([128, 2, SC], F32, tag="s2")
                        nc.tensor.matmul(
                            s2[:, 0, :],
                            kc[0:64, kcols],
                            Q_ch[c0][0:64, p, :],
                            start=True, stop=True,
                        )
                        nc.tensor.matmul(
                            s2[:, 1, :],
                            kc[64:128, kcols],
                            Q_ch[c0][64:128, p, :],
                            start=True, stop=True,
                            tile_position=(64, 0),
                        )
                        # on diagonal blocks, only q-cols >= q0 are live
                        j = kb - (nkb - 4)
                        q0 = 128 * j if j >= 0 else 0
                        w = SC - q0
                        p2 = p_pool.tile([128, 2, SC], BF16, tag="p2")
                        nc.scalar.activation(p2[:, :, q0:], s2[:, :, q0:],
                                             EXP, scale=SCALE)
                        if j >= 0:
                            msk = mask_sb[:, 384: 896 - q0]
                            nc.vector.tensor_tensor(
                                p2[:, :, q0:], p2[:, :, q0:],
                                msk.unsqueeze(1).to_broadcast([128, 2, w]), MUL)
                        nc.tensor.matmul(
                            o0[:, q0:], V1_kb[kb][:, 0:65], p2[:, 0, q0:],
                            start=(kb == 0), stop=(kb == nkb - 1),
                        )
                        nc.tensor.matmul(
                            o1[:, q0:], V1_kb[kb][:, 0:65], p2[:, 1, q0:],
                            start=(kb == 0), stop=(kb == nkb - 1),
                        )
                    # evacuate PSUM accumulator to SBUF (frees the banks)
                    oc2 = oc_pool.tile([65, 2, SC], F32, tag="oc")
                    nc.vector.tensor_copy(oc2[:], op2[:])
                    divide_pair(a_pair[p], oc2)
                return a_pair

            def divide_pair(a_tile, oc2):
                # reshape each [1, 512] sums row to [32, 16] so the
                # reciprocal runs on many DVE lanes instead of one
                rsum = r_pool.tile([64, 16], F32, tag="rsum")
                for hh in range(2):
                    nc.sync.dma_start(
                        rsum[32 * hh: 32 * hh + 32, :],
                        oc2[64:65, hh, :].rearrange("o (a n) -> o a n", a=32))
                rrecs = r_pool.tile([64, 16], F32R, tag="rrecs")
                with nc.allow_low_precision(reason="f32r-typed reciprocal output"):
                    nc.vector.reciprocal(rrecs[:], rsum[:])
                rrec = r_pool.tile([1, 2, SC], F32R, tag="rrec")
                for hh in range(2):
                    nc.sync.dma_start(
                        rrec[0:1, hh, :].rearrange("o (a n) -> o a n", a=32),
                        rrecs[32 * hh: 32 * hh + 32, :])
                for hh in range(2):
                    bc = acc_pool.tile([64, SC], F32, tag="acc")
                    nc.tensor.matmul(
                        bc[:], ones1[0:1, :], rrec[0:1, hh, :],
                        start=True, stop=True,
                    )
                    if hh == 0:
                        nc.vector.tensor_tensor(
                            a_tile[0:64, :], oc2[0:64, hh, :], bc[:], MUL)
                    else:
                        tb = r_pool.tile([64, SC], BF16, tag="tb")
                        nc.vector.tensor_tensor(
                            tb[:], oc2[0:64, hh, :], bc[:], MUL)
                        # move to partitions 64:128 (DMA crosses partitions)
                        nc.sync.dma_start(a_tile[64:128, :], tb[:])

            def out_proj(c0):
                ncols = slice(c0 * SC, (c0 + 1) * SC)
                for mb in range(8):
                    o_ps = acc_pool.tile([128, SC], F32, tag="acc")
                    for c in range(2):
                        nc.tensor.matmul(
                            o_ps[:],
                            wo_sb[:, c, mb * 128:(mb + 1) * 128],
                            A_ch[c0][c][:, :],
                            start=(c == 0), stop=(c == 1),
                        )
                    osb = o_pool.tile([128, SC], F32, tag="osb")
                    nc.vector.tensor_copy(osb[:], o_ps[:])
                    nc.sync.dma_start(out_v[:, mb, ncols], osb[:])

            proj(0)
            for c0 in range(NCH):
                attention(c0)
                if c0 + 1 < NCH:
                    proj(c0 + 1)       # fills the PE while division runs
                out_proj(c0)

    nc.compile()
    return nc


def shard_inputs(x, wq, wk, wv, wo, freqs_cos, freqs_sin):
    """Build the 8 per-core input maps (host-side layout prep)."""
    x = np.ascontiguousarray(np.asarray(x, dtype=np.float32))
    wq = np.asarray(wq, dtype=np.float32)
    wk = np.asarray(wk, dtype=np.float32)
    wv = np.asarray(wv, dtype=np.float32)
    wo = np.asarray(wo, dtype=np.float32)
    cos = np.asarray(freqs_cos, dtype=np.float32)   # [S, 32]
    sin = np.asarray(freqs_sin, dtype=np.float32)

    rope_cos = np.repeat(cos.T, 2, axis=0)          # [64, S]
    rope_sin = np.repeat(sin.T, 2, axis=0)
    rope_sin[0::2, :] *= -1.0                       # row 2i: -sin_i, 2i+1: +sin_i

    # causal keep-mask M[k, c] = 1 iff k <= c - 384; tiles slice cols
    kk = np.arange(128)[:, None]
    cc = np.arange(896)[None, :]
    mask01 = (kk <= cc - 384).astype(ml_dtypes.bfloat16)

    in_maps = []
    for core in range(N_CORES):
        b, r = divmod(core, TPG)
        xT = np.ascontiguousarray(x[b].T)                         # [DM, S]
        wq_s = wq[r * KFEAT:(r + 1) * KFEAT]                      # [256, DM]
        wk_s = wk[r * HD:(r + 1) * HD]                            # [64, DM]
        wv_s = wv[r * HD:(r + 1) * HD]
        wkvT = np.ascontiguousarray(
            np.concatenate([wk_s, wv_s], axis=0).T)               # [DM, 128]
        wqT = np.ascontiguousarray(wq_s.T)                        # [DM, 256]
        woT = np.ascontiguousarray(wo[:, r * KFEAT:(r + 1) * KFEAT].T)  # [256, DM]
        bf = ml_dtypes.bfloat16
        in_maps.append({
            "xT": xT.astype(bf),
            "wqT": wqT.astype(bf),
            "wkvT": wkvT.astype(bf),
            "woT": woT.astype(bf),
            "ropeCos": rope_cos,
            "ropeSin": rope_sin,
            "mask01": mask01,
        })
    return in_maps


def unshard(results):
    """Sum TP partials per batch and transpose back to [B, S, DM]."""
    out = np.empty((B, S, DM), dtype=np.float32)
    for b in range(B):
        acc = results[b * TPG]["out"].astype(np.float32).copy()
        for r in range(1, TPG):
            acc += results[b * TPG + r]["out"]
        out[b] = acc.T
    return out


def kernel(**inputs):
    global LAST_RESULTS, _NC_CACHE
    if _NC_CACHE is None:
        _NC_CACHE = build_nc()
    in_maps = shard_inputs(**inputs)
    LAST_RESULTS = run_bass_kernel_spmd(_NC_CACHE, in_maps, list(range(N_CORES)))
    return unshard(LAST_RESULTS.results)


# revision 27
# speedup vs baseline: 1.0151x; 1.0013x over previous
"""GQA attention (B=2, S=2048, DM=1024, H=16, KH=4, RoPE, causal) on 8 TRN2 cores.

Sharding: DP=2 over batch x TP=4 over heads. Core c handles batch c//4 and
q-heads [4r, 4r+4), kv-head r, where r = c % 4. Each core computes a partial
out^T = wo_shard @ attn_shard of shape [DM, S]; the host sums the 4 partials
per batch and transposes (gather/unshard).

Per-core kernel (single NEFF, SPMD):
  - Q/K computed feature-major ([feat, tok]) via transposed weight layouts
    prepared on host; V computed feature-major then PE-transposed to
    token-major with a ones-column appended (rowsum trick).
  - RoPE: adjacent-partition swap via stream_shuffle + elementwise muls with
    replicated cos / (+-)sin tables.
  - Causal softmax without max-subtraction (logits are bounded ~|2.5| for
    this distribution); exp on ACT engine over two PSUM banks at once
    (both heads of a pair); causal masking as a post-exp bf16 multiply on
    the 4 diagonal key-blocks only.
  - Scores matmuls run two heads concurrently in the two 64-row PE groups
    (K duplicated to partitions 64..127).
  - All dense matmuls in float32r (full PE rate at free-dim 512); AV in bf16.
"""

import numpy as np
import ml_dtypes

import concourse.bass as bass
import concourse.mybir as mybir
import concourse.tile as tile
from concourse import bacc
from concourse.bass_utils import run_bass_kernel_spmd
from concourse.masks import make_identity

F32 = mybir.dt.float32
F32R = mybir.dt.float32r
BF16 = mybir.dt.bfloat16

B, S, DM, H, KH, HD = 2, 2048, 1024, 16, 4, 64
N_CORES = 8
TPG = 4                 # tensor-parallel group size
QH = H // TPG           # q-heads per core
KFEAT = QH * HD         # 256 q-features per core
SC = 512                # token chunk
NCH = S // SC           # 4
KB = 128                # key block
NKB = S // KB           # 16
SCALE = 1.0 / np.sqrt(HD)
XOR1 = [i ^ 1 for i in range(32)]

LAST_RESULTS = None     # BassKernelResults of the most recent run (for test.py)
_NC_CACHE = None


def build_nc():
    nc = bacc.Bacc("TRN2", target_bir_lowering=False, debug=False,
                   num_devices=N_CORES)

    xT = nc.declare_dram_parameter("xT", [DM, S], BF16, isOutput=False)
    wqT = nc.declare_dram_parameter("wqT", [DM, KFEAT], BF16, isOutput=False)
    wkvT = nc.declare_dram_parameter("wkvT", [DM, 128], BF16, isOutput=False)
    woT = nc.declare_dram_parameter("woT", [KFEAT, DM], BF16, isOutput=False)
    ropeCos = nc.declare_dram_parameter("ropeCos", [64, S], F32, isOutput=False)
    ropeSin = nc.declare_dram_parameter("ropeSin", [64, S], F32, isOutput=False)
    mask01 = nc.declare_dram_parameter("mask01", [128, 896], BF16, isOutput=False)
    out = nc.declare_dram_parameter("out", [DM, S], F32, isOutput=True)

    xT_v = xT.rearrange("(kb p) n -> p kb n", p=128)        # [128, 8, S]
    wqT_v = wqT.rearrange("(kb p) m -> p kb m", p=128)      # [128, 8, 256]
    wkvT_v = wkvT.rearrange("(kb p) m -> p kb m", p=128)    # [128, 8, 128]
    woT_v = woT.rearrange("(c p) n -> p c n", p=128)        # [128, 2, 1024]
    out_v = out.rearrange("(mb p) n -> p mb n", p=128)      # [128, 8, S]

    EXP = mybir.ActivationFunctionType.Exp
    MUL = bass.mybir.AluOpType.mult
    ADD = bass.mybir.AluOpType.add

    with tile.TileContext(nc) as tc:
        with (
            tc.tile_pool(name="consts", bufs=1) as consts,
            tc.tile_pool(name="kch", bufs=NCH) as kch_pool,
            tc.tile_pool(name="qch", bufs=NCH) as qch_pool,
            tc.tile_pool(name="ach", bufs=NCH) as ach_pool,
            tc.tile_pool(name="v1p", bufs=NKB) as v1_pool,
            tc.tile_pool(name="xch", bufs=2) as xch_pool,
            tc.tile_pool(name="tmp", bufs=2) as tmp_pool,
            tc.tile_pool(name="pp", bufs=5) as p_pool,
            tc.tile_pool(name="rp", bufs=2) as r_pool,
            tc.tile_pool(name="op", bufs=3) as o_pool,
            tc.tile_pool(name="ocp", bufs=6) as oc_pool,
            tc.tile_pool(name="acc", bufs=2, space="PSUM") as acc_pool,
            tc.tile_pool(name="oac", bufs=1, space="PSUM") as oacc_pool,
            tc.tile_pool(name="sme", bufs=2, space="PSUM") as s_pool,
        ):
            # ---- constants ----
            wq_sb = consts.tile([128, 8, KFEAT], BF16, tag="wq")
            wkv_sb = consts.tile([128, 8, 128], BF16, tag="wkv")
            wo_sb = consts.tile([128, 2, DM], BF16, tag="wo")
            cos_sb = consts.tile([128, S], F32, tag="cos")
            sin_sb = consts.tile([128, S], F32, tag="sin")
            mask_sb = consts.tile([128, 896], BF16, tag="mask")
            ident = consts.tile([128, 128], BF16, tag="ident")

            nc.sync.dma_start(wq_sb[:], wqT_v)
            nc.sync.dma_start(wkv_sb[:], wkvT_v)
            nc.sync.dma_start(wo_sb[:], woT_v)
            nc.sync.dma_start(cos_sb[0:64, :], ropeCos[:])
            nc.sync.dma_start(cos_sb[64:128, :], ropeCos[:])
            nc.sync.dma_start(sin_sb[0:64, :], ropeSin[:])
            nc.sync.dma_start(sin_sb[64:128, :], ropeSin[:])
            nc.sync.dma_start(mask_sb[:], mask01[:])
            make_identity(nc, ident[:])
            ones1f = consts.tile([1, 64], F32, tag="ones1f")
            ones1 = consts.tile([1, 64], F32R, tag="ones1")
            nc.vector.memset(ones1f[:], 1.0)
            nc.vector.tensor_copy(ones1[:], ones1f[:])

            K_ch = []       # per-chunk K, feature-major, duplicated rows
            Q_ch = []       # per-chunk Q, feature-major, [128, 2, SC]
            A_ch = []       # per-chunk attn output, feature-major
            V1_kb = []      # per key-block token-major [V | 1]

            def proj(c0):
                cols = slice(c0 * SC, (c0 + 1) * SC)
                x_sb = xch_pool.tile([128, 8, SC], BF16, tag="x")
                nc.sync.dma_start(x_sb[:], xT_v[:, :, cols])

                q_sb = qch_pool.tile([128, 2, SC], BF16, tag="q")
                k_sb = kch_pool.tile([128, SC], BF16, tag="k")
                Q_ch.append(q_sb)
                K_ch.append(k_sb)

                # Q projection + RoPE, two 128-feature tiles (2 heads each)
                for m in range(2):
                    q_ps = acc_pool.tile([128, SC], F32, tag="acc")
                    for kb in range(8):
                        nc.tensor.matmul(
                            q_ps[:],
                            wq_sb[:, kb, m * 128:(m + 1) * 128],
                            x_sb[:, kb, :],
                            start=(kb == 0), stop=(kb == 7),
                        )
                    qsw = tmp_pool.tile([128, SC], F32, tag="qsw")
                    t1 = tmp_pool.tile([128, SC], F32, tag="t1")
                    t2 = tmp_pool.tile([128, SC], F32, tag="t2")
                    nc.vector.stream_shuffle(qsw[:], q_ps[:], XOR1)
                    nc.vector.tensor_tensor(t1[:], q_ps[:], cos_sb[:, cols], MUL)
                    nc.vector.tensor_tensor(t2[:], qsw[:], sin_sb[:, cols], MUL)
                    nc.vector.tensor_tensor(q_sb[:, m, :], t1[:], t2[:], ADD)

                # K (rows 0:64) and V (rows 64:128) projection
                kv_ps = acc_pool.tile([128, SC], F32, tag="acc")
                for kb in range(8):
                    nc.tensor.matmul(
                        kv_ps[:],
                        wkv_sb[:, kb, :],
                        x_sb[:, kb, :],
                        start=(kb == 0), stop=(kb == 7),
                    )
                # K RoPE
                ksw = tmp_pool.tile([64, SC], F32, tag="ksw")
                t1k = tmp_pool.tile([64, SC], F32, tag="t1k")
                t2k = tmp_pool.tile([64, SC], F32, tag="t2k")
                nc.vector.stream_shuffle(ksw[:], kv_ps[0:64, :], XOR1)
                nc.vector.tensor_tensor(t1k[:], kv_ps[0:64, :], cos_sb[0:64, cols], MUL)
                nc.vector.tensor_tensor(t2k[:], ksw[:], sin_sb[0:64, cols], MUL)
                nc.vector.tensor_tensor(k_sb[0:64, :], t1k[:], t2k[:], ADD)
                # duplicate K into partitions 64:128 (second PE row group)
                nc.sync.dma_start(k_sb[64:128, :], k_sb[0:64, :])

                # V: cast to bf16, PE-transpose to token-major, append ones col
                vtmp = tmp_pool.tile([128, SC], BF16, tag="vtmp")
                nc.vector.tensor_copy(vtmp[64:128, :], kv_ps[64:128, :])
                for tb in range(4):
                    v1 = v1_pool.tile([128, 66], BF16, tag="v1")
                    V1_kb.append(v1)
                    vt_ps = acc_pool.tile([128, 64], BF16, tag="acc")
                    nc.tensor.transpose(
                        vt_ps[:], vtmp[64:128, tb * 128:(tb + 1) * 128],
                        ident[64:128, 64:128],
                    )
                    nc.vector.tensor_copy(v1[:, 0:64], vt_ps[:])
                    nc.vector.memset(v1[:, 64:65], 1.0)

            def attention(c0):
                nkb = 4 * (c0 + 1)
                a_pair = [ach_pool.tile([128, SC], BF16, tag="a",
                                        name=f"a_c{c0}p{i}")
                          for i in range(2)]
                A_ch.append(a_pair)
                oc_list = []
                for p in range(2):
                    op2 = oacc_pool.tile([65, 2, SC], F32, tag="oacc")
                    o0 = op2[:, 0, :]
                    o1 = op2[:, 1, :]
                    for kb in range(nkb):
                        kc = K_ch[kb // 4]
                        kcols = slice((kb % 4) * 128, (kb % 4 + 1) * 128)
                        s2 = s_pool.tile([128, 2, SC], F32, tag="s2")
                        nc.tensor.matmul(
                            s2[:, 0, :],
                            kc[0:64, kcols],
                            Q_ch[c0][0:64, p, :],
                            start=True, stop=True,
                        )
                        nc.tensor.matmul(
                            s2[:, 1, :],
                            kc[64:128, kcols],
                            Q_ch[c0][64:128, p, :],
                            start=True, stop=True,
                            tile_position=(64, 0),
                        )
                        # on diagonal blocks, only q-cols >= q0 are live
                        j = kb - (nkb - 4)
                        q0 = 128 * j if j >= 0 else 0
                        w = SC - q0
                        p2 = p_pool.tile([128, 2, SC], BF16, tag="p2")
                        nc.scalar.activation(p2[:, :, q0:], s2[:, :, q0:],
                                             EXP, scale=SCALE)
                        if j >= 0:
                            msk = mask_sb[:, 384: 896 - q0]
                            nc.vector.tensor_tensor(
                                p2[:, :, q0:], p2[:, :, q0:],
                                msk.unsqueeze(1).to_broadcast([128, 2, w]), MUL)
                        nc.tensor.matmul(
                            o0[:, q0:], V1_kb[kb][:, 0:65], p2[:, 0, q0:],
                            start=(kb == 0), stop=(kb == nkb - 1),
                        )
                        nc.tensor.matmul(
                            o1[:, q0:], V1_kb[kb][:, 0:65], p2[:, 1, q0:],
                            start=(kb == 0), stop=(kb == nkb - 1),
                        )
                    # evacuate PSUM accumulator to SBUF (frees the banks)
                    oc2 = oc_pool.tile([65, 2, SC], F32, tag="oc")
                    nc.vector.tensor_copy(oc2[:], op2[:])
                    oc_list.append(oc2)
                for p in range(2):
                    divide_pair(a_pair[p], oc_list[p])
                return a_pair

            def divide_pair(a_tile, oc2):
                # reshape each [1, 512] sums row to [32, 16] so the
                # reciprocal runs on many DVE lanes instead of one
                rsum = r_pool.tile([64, 16], F32, tag="rsum")
                for hh in range(2):
                    nc.sync.dma_start(
                        rsum[32 * hh: 32 * hh + 32, :],
                        oc2[64:65, hh, :].rearrange("o (a n) -> o a n", a=32))
                rrecs = r_pool.tile([64, 16], F32R, tag="rrecs")
                with nc.allow_low_precision(reason="f32r-typed reciprocal output"):
                    nc.vector.reciprocal(rrecs[:], rsum[:])
                rrec = r_pool.tile([1, 2, SC], F32R, tag="rrec")
                for hh in range(2):
                    nc.sync.dma_start(
                        rrec[0:1, hh, :].rearrange("o (a n) -> o a n", a=32),
                        rrecs[32 * hh: 32 * hh + 32, :])
                for hh in range(2):
                    bc = acc_pool.tile([64, SC], F32, tag="acc")
                    nc.tensor.matmul(
                        bc[:], ones1[0:1, :], rrec[0:1, hh, :],
                        start=True, stop=True,
                    )
                    if hh == 0:
                        nc.vector.tensor_tensor(
                            a_tile[0:64, :], oc2[0:64, hh, :], bc[:], MUL)
                    else:
                        tb = r_pool.tile([64, SC], BF16, tag="tb")
                        nc.vector.tensor_tensor(
                            tb[:], oc2[0:64, hh, :], bc[:], MUL)
                        # move to partitions 64:128 (DMA crosses partitions)
                        nc.sync.dma_start(a_tile[64:128, :], tb[:])

            def out_proj(c0):
                ncols = slice(c0 * SC, (c0 + 1) * SC)
                for mb in range(8):
                    o_ps = acc_pool.tile([128, SC], F32, tag="acc")
                    for c in range(2):
                        nc.tensor.matmul(
                            o_ps[:],
                            wo_sb[:, c, mb * 128:(mb + 1) * 128],
                            A_ch[c0][c][:, :],
                            start=(c == 0), stop=(c == 1),
                        )
                    osb = o_pool.tile([128, SC], F32, tag="osb")
                    nc.vector.tensor_copy(osb[:], o_ps[:])
                    nc.sync.dma_start(out_v[:, mb, ncols], osb[:])

            proj(0)
            for c0 in range(NCH):
                attention(c0)
                if c0 + 1 < NCH:
                    proj(c0 + 1)       # fills the PE while division runs
                out_proj(c0)

    nc.compile()
    return nc


def shard_inputs(x, wq, wk, wv, wo, freqs_cos, freqs_sin):
    """Build the 8 per-core input maps (host-side layout prep)."""
    x = np.ascontiguousarray(np.asarray(x, dtype=np.float32))
    wq = np.asarray(wq, dtype=np.float32)
    wk = np.asarray(wk, dtype=np.float32)
    wv = np.asarray(wv, dtype=np.float32)
    wo = np.asarray(wo, dtype=np.float32)
    cos = np.asarray(freqs_cos, dtype=np.float32)   # [S, 32]
    sin = np.asarray(freqs_sin, dtype=np.float32)

    rope_cos = np.repeat(cos.T, 2, axis=0)          # [64, S]
    rope_sin = np.repeat(sin.T, 2, axis=0)
    rope_sin[0::2, :] *= -1.0                       # row 2i: -sin_i, 2i+1: +sin_i

    # causal keep-mask M[k, c] = 1 iff k <= c - 384; tiles slice cols
    kk = np.arange(128)[:, None]
    cc = np.arange(896)[None, :]
    mask01 = (kk <= cc - 384).astype(ml_dtypes.bfloat16)

    in_maps = []
    for core in range(N_CORES):
        b, r = divmod(core, TPG)
        xT = np.ascontiguousarray(x[b].T)                         # [DM, S]
        wq_s = wq[r * KFEAT:(r + 1) * KFEAT]                      # [256, DM]
        wk_s = wk[r * HD:(r + 1) * HD]                            # [64, DM]
        wv_s = wv[r * HD:(r + 1) * HD]
        wkvT = np.ascontiguousarray(
            np.concatenate([wk_s, wv_s], axis=0).T)               # [DM, 128]
        wqT = np.ascontiguousarray(wq_s.T)                        # [DM, 256]
        woT = np.ascontiguousarray(wo[:, r * KFEAT:(r + 1) * KFEAT].T)  # [256, DM]
        bf = ml_dtypes.bfloat16
        in_maps.append({
            "xT": xT.astype(bf),
            "wqT": wqT.astype(bf),
            "wkvT": wkvT.astype(bf),
            "woT": woT.astype(bf),
            "ropeCos": rope_cos,
            "ropeSin": rope_sin,
            "mask01": mask01,
        })
    return in_maps


def unshard(results):
    """Sum TP partials per batch and transpose back to [B, S, DM]."""
    out = np.empty((B, S, DM), dtype=np.float32)
    for b in range(B):
        acc = results[b * TPG]["out"].astype(np.float32).copy()
        for r in range(1, TPG):
            acc += results[b * TPG + r]["out"]
        out[b] = acc.T
    return out


def kernel(**inputs):
    global LAST_RESULTS, _NC_CACHE
    if _NC_CACHE is None:
        _NC_CACHE = build_nc()
    in_maps = shard_inputs(**inputs)
    LAST_RESULTS = run_bass_kernel_spmd(_NC_CACHE, in_maps, list(range(N_CORES)))
    return unshard(LAST_RESULTS.results)


# revision 28
# speedup vs baseline: 1.0163x; 1.0011x over previous
"""GQA attention (B=2, S=2048, DM=1024, H=16, KH=4, RoPE, causal) on 8 TRN2 cores.

Sharding: DP=2 over batch x TP=4 over heads. Core c handles batch c//4 and
q-heads [4r, 4r+4), kv-head r, where r = c % 4. Each core computes a partial
out^T = wo_shard @ attn_shard of shape [DM, S]; the host sums the 4 partials
per batch and transposes (gather/unshard).

Per-core kernel (single NEFF, SPMD):
  - Q/K computed feature-major ([feat, tok]) via transposed weight layouts
    prepared on host; V computed feature-major then PE-transposed to
    token-major with a ones-column appended (rowsum trick).
  - RoPE: adjacent-partition swap via stream_shuffle + elementwise muls with
    replicated cos / (+-)sin tables.
  - Causal softmax without max-subtraction (logits are bounded ~|2.5| for
    this distribution); exp on ACT engine over two PSUM banks at once
    (both heads of a pair); causal masking as a post-exp bf16 multiply on
    the 4 diagonal key-blocks only.
  - Scores matmuls run two heads concurrently in the two 64-row PE groups
    (K duplicated to partitions 64..127).
  - All dense matmuls in float32r (full PE rate at free-dim 512); AV in bf16.
"""

import numpy as np
import ml_dtypes

import concourse.bass as bass
import concourse.mybir as mybir
import concourse.tile as tile
from concourse import bacc
from concourse.bass_utils import run_bass_kernel_spmd
from concourse.masks import make_identity

F32 = mybir.dt.float32
F32R = mybir.dt.float32r
BF16 = mybir.dt.bfloat16

B, S, DM, H, KH, HD = 2, 2048, 1024, 16, 4, 64
N_CORES = 8
TPG = 4                 # tensor-parallel group size
QH = H // TPG           # q-heads per core
KFEAT = QH * HD         # 256 q-features per core
SC = 512                # token chunk
NCH = S // SC           # 4
KB = 128                # key block
NKB = S // KB           # 16
SCALE = 1.0 / np.sqrt(HD)
XOR1 = [i ^ 1 for i in range(32)]

LAST_RESULTS = None     # BassKernelResults of the most recent run (for test.py)
_NC_CACHE = None


def build_nc():
    nc = bacc.Bacc("TRN2", target_bir_lowering=False, debug=False,
                   num_devices=N_CORES)

    xT = nc.declare_dram_parameter("xT", [DM, S], BF16, isOutput=False)
    wqT = nc.declare_dram_parameter("wqT", [DM, KFEAT], BF16, isOutput=False)
    wkvT = nc.declare_dram_parameter("wkvT", [DM, 128], BF16, isOutput=False)
    woT = nc.declare_dram_parameter("woT", [KFEAT, DM], BF16, isOutput=False)
    ropeCos = nc.declare_dram_parameter("ropeCos", [64, S], F32, isOutput=False)
    ropeSin = nc.declare_dram_parameter("ropeSin", [64, S], F32, isOutput=False)
    mask01 = nc.declare_dram_parameter("mask01", [128, 896], BF16, isOutput=False)
    out = nc.declare_dram_parameter("out", [DM, S], F32, isOutput=True)

    xT_v = xT.rearrange("(kb p) n -> p kb n", p=128)        # [128, 8, S]
    wqT_v = wqT.rearrange("(kb p) m -> p kb m", p=128)      # [128, 8, 256]
    wkvT_v = wkvT.rearrange("(kb p) m -> p kb m", p=128)    # [128, 8, 128]
    woT_v = woT.rearrange("(c p) n -> p c n", p=128)        # [128, 2, 1024]
    out_v = out.rearrange("(mb p) n -> p mb n", p=128)      # [128, 8, S]

    EXP = mybir.ActivationFunctionType.Exp
    MUL = bass.mybir.AluOpType.mult
    ADD = bass.mybir.AluOpType.add

    with tile.TileContext(nc) as tc:
        with (
            tc.tile_pool(name="consts", bufs=1) as consts,
            tc.tile_pool(name="kch", bufs=NCH) as kch_pool,
            tc.tile_pool(name="qch", bufs=NCH) as qch_pool,
            tc.tile_pool(name="ach", bufs=2 * NCH) as ach_pool,
            tc.tile_pool(name="v1p", bufs=NKB) as v1_pool,
            tc.tile_pool(name="xch", bufs=2) as xch_pool,
            tc.tile_pool(name="tmp", bufs=2) as tmp_pool,
            tc.tile_pool(name="pp", bufs=5) as p_pool,
            tc.tile_pool(name="rp", bufs=2) as r_pool,
            tc.tile_pool(name="op", bufs=3) as o_pool,
            tc.tile_pool(name="ocp", bufs=6) as oc_pool,
            tc.tile_pool(name="acc", bufs=2, space="PSUM") as acc_pool,
            tc.tile_pool(name="oac", bufs=1, space="PSUM") as oacc_pool,
            tc.tile_pool(name="sme", bufs=2, space="PSUM") as s_pool,
        ):
            # ---- constants ----
            wq_sb = consts.tile([128, 8, KFEAT], BF16, tag="wq")
            wkv_sb = consts.tile([128, 8, 128], BF16, tag="wkv")
            wo_sb = consts.tile([128, 2, DM], BF16, tag="wo")
            cos_sb = consts.tile([128, S], F32, tag="cos")
            sin_sb = consts.tile([128, S], F32, tag="sin")
            mask_sb = consts.tile([128, 896], BF16, tag="mask")
            ident = consts.tile([128, 128], BF16, tag="ident")

            nc.sync.dma_start(wq_sb[:], wqT_v)
            nc.sync.dma_start(wkv_sb[:], wkvT_v)
            nc.sync.dma_start(wo_sb[:], woT_v)
            nc.sync.dma_start(cos_sb[0:64, :], ropeCos[:])
            nc.sync.dma_start(cos_sb[64:128, :], ropeCos[:])
            nc.sync.dma_start(sin_sb[0:64, :], ropeSin[:])
            nc.sync.dma_start(sin_sb[64:128, :], ropeSin[:])
            nc.sync.dma_start(mask_sb[:], mask01[:])
            make_identity(nc, ident[:])
            ones1f = consts.tile([1, 64], F32, tag="ones1f")
            ones1 = consts.tile([1, 64], F32R, tag="ones1")
            nc.vector.memset(ones1f[:], 1.0)
            nc.vector.tensor_copy(ones1[:], ones1f[:])

            K_ch = []       # per-chunk K, feature-major, duplicated rows
            Q_ch = []       # per-chunk Q, feature-major, [128, 2, SC]
            A_ch = []       # per-chunk attn output, feature-major
            V1_kb = []      # per key-block token-major [V | 1]

            def proj(c0):
                cols = slice(c0 * SC, (c0 + 1) * SC)
                x_sb = xch_pool.tile([128, 8, SC], BF16, tag="x")
                nc.sync.dma_start(x_sb[:], xT_v[:, :, cols])

                q_sb = qch_pool.tile([128, 2, SC], BF16, tag="q")
                k_sb = kch_pool.tile([128, SC], BF16, tag="k")
                Q_ch.append(q_sb)
                K_ch.append(k_sb)

                # Q projection + RoPE, two 128-feature tiles (2 heads each)
                for m in range(2):
                    q_ps = acc_pool.tile([128, SC], F32, tag="acc")
                    for kb in range(8):
                        nc.tensor.matmul(
                            q_ps[:],
                            wq_sb[:, kb, m * 128:(m + 1) * 128],
                            x_sb[:, kb, :],
                            start=(kb == 0), stop=(kb == 7),
                        )
                    qsw = tmp_pool.tile([128, SC], F32, tag="qsw")
                    t1 = tmp_pool.tile([128, SC], F32, tag="t1")
                    t2 = tmp_pool.tile([128, SC], F32, tag="t2")
                    nc.vector.stream_shuffle(qsw[:], q_ps[:], XOR1)
                    nc.vector.tensor_tensor(t1[:], q_ps[:], cos_sb[:, cols], MUL)
                    nc.vector.tensor_tensor(t2[:], qsw[:], sin_sb[:, cols], MUL)
                    nc.vector.tensor_tensor(q_sb[:, m, :], t1[:], t2[:], ADD)

                # K (rows 0:64) and V (rows 64:128) projection
                kv_ps = acc_pool.tile([128, SC], F32, tag="acc")
                for kb in range(8):
                    nc.tensor.matmul(
                        kv_ps[:],
                        wkv_sb[:, kb, :],
                        x_sb[:, kb, :],
                        start=(kb == 0), stop=(kb == 7),
                    )
                # K RoPE
                ksw = tmp_pool.tile([64, SC], F32, tag="ksw")
                t1k = tmp_pool.tile([64, SC], F32, tag="t1k")
                t2k = tmp_pool.tile([64, SC], F32, tag="t2k")
                nc.vector.stream_shuffle(ksw[:], kv_ps[0:64, :], XOR1)
                nc.vector.tensor_tensor(t1k[:], kv_ps[0:64, :], cos_sb[0:64, cols], MUL)
                nc.vector.tensor_tensor(t2k[:], ksw[:], sin_sb[0:64, cols], MUL)
                nc.vector.tensor_tensor(k_sb[0:64, :], t1k[:], t2k[:], ADD)
                # duplicate K into partitions 64:128 (second PE row group)
                nc.sync.dma_start(k_sb[64:128, :], k_sb[0:64, :])

                # V: cast to bf16, PE-transpose to token-major, append ones col
                vtmp = tmp_pool.tile([128, SC], BF16, tag="vtmp")
                nc.vector.tensor_copy(vtmp[64:128, :], kv_ps[64:128, :])
                for tb in range(4):
                    v1 = v1_pool.tile([128, 66], BF16, tag="v1")
                    V1_kb.append(v1)
                    vt_ps = acc_pool.tile([128, 64], BF16, tag="acc")
                    nc.tensor.transpose(
                        vt_ps[:], vtmp[64:128, tb * 128:(tb + 1) * 128],
                        ident[64:128, 64:128],
                    )
                    nc.vector.tensor_copy(v1[:, 0:64], vt_ps[:])
                    nc.vector.memset(v1[:, 64:65], 1.0)

            def attention(c0):
                nkb = 4 * (c0 + 1)
                a_pair = [ach_pool.tile([128, SC], BF16, tag="a",
                                        name=f"a_c{c0}p{i}")
                          for i in range(2)]
                A_ch.append(a_pair)
                oc_list = []
                for p in range(2):
                    op2 = oacc_pool.tile([65, 2, SC], F32, tag="oacc")
                    o0 = op2[:, 0, :]
                    o1 = op2[:, 1, :]
                    for kb in range(nkb):
                        kc = K_ch[kb // 4]
                        kcols = slice((kb % 4) * 128, (kb % 4 + 1) * 128)
                        s2 = s_pool.tile([128, 2, SC], F32, tag="s2")
                        nc.tensor.matmul(
                            s2[:, 0, :],
                            kc[0:64, kcols],
                            Q_ch[c0][0:64, p, :],
                            start=True, stop=True,
                        )
                        nc.tensor.matmul(
                            s2[:, 1, :],
                            kc[64:128, kcols],
                            Q_ch[c0][64:128, p, :],
                            start=True, stop=True,
                            tile_position=(64, 0),
                        )
                        # on diagonal blocks, only q-cols >= q0 are live
                        j = kb - (nkb - 4)
                        q0 = 128 * j if j >= 0 else 0
                        w = SC - q0
                        p2 = p_pool.tile([128, 2, SC], BF16, tag="p2")
                        nc.scalar.activation(p2[:, :, q0:], s2[:, :, q0:],
                                             EXP, scale=SCALE)
                        if j >= 0:
                            msk = mask_sb[:, 384: 896 - q0]
                            nc.vector.tensor_tensor(
                                p2[:, :, q0:], p2[:, :, q0:],
                                msk.unsqueeze(1).to_broadcast([128, 2, w]), MUL)
                        nc.tensor.matmul(
                            o0[:, q0:], V1_kb[kb][:, 0:65], p2[:, 0, q0:],
                            start=(kb == 0), stop=(kb == nkb - 1),
                        )
                        nc.tensor.matmul(
                            o1[:, q0:], V1_kb[kb][:, 0:65], p2[:, 1, q0:],
                            start=(kb == 0), stop=(kb == nkb - 1),
                        )
                    # evacuate PSUM accumulator to SBUF (frees the banks)
                    oc2 = oc_pool.tile([65, 2, SC], F32, tag="oc")
                    nc.vector.tensor_copy(oc2[:], op2[:])
                    oc_list.append(oc2)
                for p in range(2):
                    divide_pair(a_pair[p], oc_list[p])
                return a_pair

            def divide_pair(a_tile, oc2):
                # reshape each [1, 512] sums row to [32, 16] so the
                # reciprocal runs on many DVE lanes instead of one
                rsum = r_pool.tile([64, 16], F32, tag="rsum")
                for hh in range(2):
                    nc.sync.dma_start(
                        rsum[32 * hh: 32 * hh + 32, :],
                        oc2[64:65, hh, :].rearrange("o (a n) -> o a n", a=32))
                rrecs = r_pool.tile([64, 16], F32R, tag="rrecs")
                with nc.allow_low_precision(reason="f32r-typed reciprocal output"):
                    nc.vector.reciprocal(rrecs[:], rsum[:])
                rrec = r_pool.tile([1, 2, SC], F32R, tag="rrec")
                for hh in range(2):
                    nc.sync.dma_start(
                        rrec[0:1, hh, :].rearrange("o (a n) -> o a n", a=32),
                        rrecs[32 * hh: 32 * hh + 32, :])
                for hh in range(2):
                    bc = acc_pool.tile([64, SC], F32, tag="acc")
                    nc.tensor.matmul(
                        bc[:], ones1[0:1, :], rrec[0:1, hh, :],
                        start=True, stop=True,
                    )
                    if hh == 0:
                        nc.vector.tensor_tensor(
                            a_tile[0:64, :], oc2[0:64, hh, :], bc[:], MUL)
                    else:
                        tb = r_pool.tile([64, SC], BF16, tag="tb")
                        nc.vector.tensor_tensor(
                            tb[:], oc2[0:64, hh, :], bc[:], MUL)
                        # move to partitions 64:128 (DMA crosses partitions)
                        nc.sync.dma_start(a_tile[64:128, :], tb[:])

            def out_proj(c0):
                ncols = slice(c0 * SC, (c0 + 1) * SC)
                for mb in range(8):
                    o_ps = acc_pool.tile([128, SC], F32, tag="acc")
                    for c in range(2):
                        nc.tensor.matmul(
                            o_ps[:],
                            wo_sb[:, c, mb * 128:(mb + 1) * 128],
                            A_ch[c0][c][:, :],
                            start=(c == 0), stop=(c == 1),
                        )
                    osb = o_pool.tile([128, SC], F32, tag="osb")
                    nc.vector.tensor_copy(osb[:], o_ps[:])
                    nc.sync.dma_start(out_v[:, mb, ncols], osb[:])

            proj(0)
            for c0 in range(NCH):
                attention(c0)
                if c0 + 1 < NCH:
                    proj(c0 + 1)       # fills the PE while division runs
                out_proj(c0)

    nc.compile()
    return nc


def shard_inputs(x, wq, wk, wv, wo, freqs_cos, freqs_sin):
    """Build the 8 per-core input maps (host-side layout prep)."""
    x = np.ascontiguousarray(np.asarray(x, dtype=np.float32))
    wq = np.asarray(wq, dtype=np.float32)
    wk = np.asarray(wk, dtype=np.float32)
    wv = np.asarray(wv, dtype=np.float32)
    wo = np.asarray(wo, dtype=np.float32)
    cos = np.asarray(freqs_cos, dtype=np.float32)   # [S, 32]
    sin = np.asarray(freqs_sin, dtype=np.float32)

    rope_cos = np.repeat(cos.T, 2, axis=0)          # [64, S]
    rope_sin = np.repeat(sin.T, 2, axis=0)
    rope_sin[0::2, :] *= -1.0                       # row 2i: -sin_i, 2i+1: +sin_i

    # causal keep-mask M[k, c] = 1 iff k <= c - 384; tiles slice cols
    kk = np.arange(128)[:, None]
    cc = np.arange(896)[None, :]
    mask01 = (kk <= cc - 384).astype(ml_dtypes.bfloat16)

    in_maps = []
    for core in range(N_CORES):
        b, r = divmod(core, TPG)
        xT = np.ascontiguousarray(x[b].T)                         # [DM, S]
        wq_s = wq[r * KFEAT:(r + 1) * KFEAT]                      # [256, DM]
        wk_s = wk[r * HD:(r + 1) * HD]                            # [64, DM]
        wv_s = wv[r * HD:(r + 1) * HD]
        wkvT = np.ascontiguousarray(
            np.concatenate([wk_s, wv_s], axis=0).T)               # [DM, 128]
        wqT = np.ascontiguousarray(wq_s.T)                        # [DM, 256]
        woT = np.ascontiguousarray(wo[:, r * KFEAT:(r + 1) * KFEAT].T)  # [256, DM]
        bf = ml_dtypes.bfloat16
        in_maps.append({
            "xT": xT.astype(bf),
            "wqT": wqT.astype(bf),
            "wkvT": wkvT.astype(bf),
            "woT": woT.astype(bf),
            "ropeCos": rope_cos,
            "ropeSin": rope_sin,
            "mask01": mask01,
        })
    return in_maps


def unshard(results):
    """Sum TP partials per batch and transpose back to [B, S, DM]."""
    out = np.empty((B, S, DM), dtype=np.float32)
    for b in range(B):
        acc = results[b * TPG]["out"].astype(np.float32).copy()
        for r in range(1, TPG):
            acc += results[b * TPG + r]["out"]
        out[b] = acc.T
    return out


def kernel(**inputs):
    global LAST_RESULTS, _NC_CACHE
    if _NC_CACHE is None:
        _NC_CACHE = build_nc()
    in_maps = shard_inputs(**inputs)
    LAST_RESULTS = run_bass_kernel_spmd(_NC_CACHE, in_maps, list(range(N_CORES)))
    return unshard(LAST_RESULTS.results)


# revision 29
# speedup vs baseline: 1.1259x; 1.1079x over previous
"""GQA attention (B=2, S=2048, DM=1024, H=16, KH=4, RoPE, causal) on 8 TRN2 cores.

Sharding: DP=2 over batch x TP=4 over heads. Core c handles batch c//4 and
q-heads [4r, 4r+4), kv-head r, where r = c % 4. Each core computes a partial
out^T = wo_shard @ attn_shard of shape [DM, S]; the host sums the 4 partials
per batch and transposes (gather/unshard).

Per-core kernel (single NEFF, SPMD):
  - Q/K computed feature-major ([feat, tok]) via transposed weight layouts
    prepared on host; V computed feature-major then PE-transposed to
    token-major with a ones-column appended (rowsum trick).
  - RoPE: adjacent-partition swap via stream_shuffle + elementwise muls with
    replicated cos / (+-)sin tables.
  - Causal softmax without max-subtraction (logits are bounded ~|2.5| for
    this distribution); exp on ACT engine over two PSUM banks at once
    (both heads of a pair); causal masking as a post-exp bf16 multiply on
    the 4 diagonal key-blocks only.
  - Scores matmuls run two heads concurrently in the two 64-row PE groups
    (K duplicated to partitions 64..127).
  - All dense matmuls in float32r (full PE rate at free-dim 512); AV in bf16.
"""

import numpy as np
import ml_dtypes

import concourse.bass as bass
import concourse.mybir as mybir
import concourse.tile as tile
from concourse import bacc
from concourse.bass_utils import run_bass_kernel_spmd
from concourse.masks import make_identity

F32 = mybir.dt.float32
F32R = mybir.dt.float32r
BF16 = mybir.dt.bfloat16

B, S, DM, H, KH, HD = 2, 2048, 1024, 16, 4, 64
N_CORES = 8
TPG = 4                 # tensor-parallel group size
QH = H // TPG           # q-heads per core
KFEAT = QH * HD         # 256 q-features per core
SC = 512                # token chunk
NCH = S // SC           # 4
KB = 128                # key block
NKB = S // KB           # 16
SCALE = 1.0 / np.sqrt(HD)
XOR1 = [i ^ 1 for i in range(32)]

LAST_RESULTS = None     # BassKernelResults of the most recent run (for test.py)
_NC_CACHE = None


def build_nc():
    nc = bacc.Bacc("TRN2", target_bir_lowering=False, debug=False,
                   num_devices=N_CORES)

    xT = nc.declare_dram_parameter("xT", [DM, S], BF16, isOutput=False)
    wqT = nc.declare_dram_parameter("wqT", [DM, KFEAT], BF16, isOutput=False)
    wkvT = nc.declare_dram_parameter("wkvT", [DM, 128], BF16, isOutput=False)
    woT = nc.declare_dram_parameter("woT", [KFEAT, DM], BF16, isOutput=False)
    ropeCos = nc.declare_dram_parameter("ropeCos", [64, S], F32, isOutput=False)
    ropeSin = nc.declare_dram_parameter("ropeSin", [64, S], F32, isOutput=False)
    mask01 = nc.declare_dram_parameter("mask01", [128, 896], BF16, isOutput=False)
    out = nc.declare_dram_parameter("out", [DM, S], F32, isOutput=True)

    xT_v = xT.rearrange("(kb p) n -> p kb n", p=128)        # [128, 8, S]
    wqT_v = wqT.rearrange("(kb p) m -> p kb m", p=128)      # [128, 8, 256]
    wkvT_v = wkvT.rearrange("(kb p) m -> p kb m", p=128)    # [128, 8, 128]
    woT_v = woT.rearrange("(c p) n -> p c n", p=128)        # [128, 2, 1024]
    out_v = out.rearrange("(mb p) n -> p mb n", p=128)      # [128, 8, S]

    EXP = mybir.ActivationFunctionType.Exp
    MUL = bass.mybir.AluOpType.mult
    ADD = bass.mybir.AluOpType.add

    with tile.TileContext(nc) as tc:
        with (
            tc.tile_pool(name="consts", bufs=1) as consts,
            tc.tile_pool(name="kch", bufs=NCH) as kch_pool,
            tc.tile_pool(name="qch", bufs=NCH) as qch_pool,
            tc.tile_pool(name="ach", bufs=2 * NCH) as ach_pool,
            tc.tile_pool(name="v1p", bufs=NKB) as v1_pool,
            tc.tile_pool(name="xch", bufs=2) as xch_pool,
            tc.tile_pool(name="tmp", bufs=2) as tmp_pool,
            tc.tile_pool(name="pp", bufs=5) as p_pool,
            tc.tile_pool(name="rp", bufs=2) as r_pool,
            tc.tile_pool(name="op", bufs=3) as o_pool,
            tc.tile_pool(name="ocp", bufs=6) as oc_pool,
            tc.tile_pool(name="acc", bufs=2, space="PSUM") as acc_pool,
            tc.tile_pool(name="oac", bufs=1, space="PSUM") as oacc_pool,
            tc.tile_pool(name="sme", bufs=2, space="PSUM") as s_pool,
        ):
            # ---- constants ----
            wq_sb = consts.tile([128, 8, KFEAT], BF16, tag="wq")
            wkv_sb = consts.tile([128, 8, 128], BF16, tag="wkv")
            wo_sb = consts.tile([128, 2, DM], BF16, tag="wo")
            cos_sb = consts.tile([128, S], F32, tag="cos")
            sin_sb = consts.tile([128, S], F32, tag="sin")
            mask_sb = consts.tile([128, 896], BF16, tag="mask")
            ident = consts.tile([128, 128], BF16, tag="ident")

            nc.sync.dma_start(wq_sb[:], wqT_v)
            nc.sync.dma_start(wkv_sb[:], wkvT_v)
            nc.sync.dma_start(wo_sb[:], woT_v)
            nc.sync.dma_start(cos_sb[0:64, :], ropeCos[:])
            nc.sync.dma_start(cos_sb[64:128, :], ropeCos[:])
            nc.sync.dma_start(sin_sb[0:64, :], ropeSin[:])
            nc.sync.dma_start(sin_sb[64:128, :], ropeSin[:])
            nc.sync.dma_start(mask_sb[:], mask01[:])
            make_identity(nc, ident[:])
            ones1f = consts.tile([1, 64], F32, tag="ones1f")
            ones1 = consts.tile([1, 64], F32R, tag="ones1")
            nc.vector.memset(ones1f[:], 1.0)
            nc.vector.tensor_copy(ones1[:], ones1f[:])

            K_ch = []       # per-chunk K, feature-major, duplicated rows
            Q_ch = []       # per-chunk Q, feature-major, [128, 2, SC]
            A_ch = []       # per-chunk attn output, feature-major
            V1_kb = []      # per key-block token-major [V | 1]

            def proj(c0):
                cols = slice(c0 * SC, (c0 + 1) * SC)
                x_sb = xch_pool.tile([128, 8, SC], BF16, tag="x")
                nc.sync.dma_start(x_sb[:], xT_v[:, :, cols])

                q_sb = qch_pool.tile([128, 2, SC], BF16, tag="q")
                k_sb = kch_pool.tile([128, SC], BF16, tag="k")
                Q_ch.append(q_sb)
                K_ch.append(k_sb)

                # Q projection + RoPE, two 128-feature tiles (2 heads each)
                for m in range(2):
                    q_ps = acc_pool.tile([128, SC], F32, tag="acc")
                    for kb in range(8):
                        nc.tensor.matmul(
                            q_ps[:],
                            wq_sb[:, kb, m * 128:(m + 1) * 128],
                            x_sb[:, kb, :],
                            start=(kb == 0), stop=(kb == 7),
                        )
                    qsw = tmp_pool.tile([128, SC], F32, tag="qsw")
                    t1 = tmp_pool.tile([128, SC], F32, tag="t1")
                    t2 = tmp_pool.tile([128, SC], F32, tag="t2")
                    nc.vector.stream_shuffle(qsw[:], q_ps[:], XOR1)
                    nc.vector.tensor_tensor(t1[:], q_ps[:], cos_sb[:, cols], MUL)
                    nc.vector.tensor_tensor(t2[:], qsw[:], sin_sb[:, cols], MUL)
                    nc.vector.tensor_tensor(q_sb[:, m, :], t1[:], t2[:], ADD)

                # K (rows 0:64) and V (rows 64:128) projection
                kv_ps = acc_pool.tile([128, SC], F32, tag="acc")
                for kb in range(8):
                    nc.tensor.matmul(
                        kv_ps[:],
                        wkv_sb[:, kb, :],
                        x_sb[:, kb, :],
                        start=(kb == 0), stop=(kb == 7),
                    )
                # K RoPE
                ksw = tmp_pool.tile([64, SC], F32, tag="ksw")
                t1k = tmp_pool.tile([64, SC], F32, tag="t1k")
                t2k = tmp_pool.tile([64, SC], F32, tag="t2k")
                nc.vector.stream_shuffle(ksw[:], kv_ps[0:64, :], XOR1)
                nc.vector.tensor_tensor(t1k[:], kv_ps[0:64, :], cos_sb[0:64, cols], MUL)
                nc.vector.tensor_tensor(t2k[:], ksw[:], sin_sb[0:64, cols], MUL)
                nc.vector.tensor_tensor(k_sb[0:64, :], t1k[:], t2k[:], ADD)
                # duplicate K into partitions 64:128 (second PE row group)
                nc.sync.dma_start(k_sb[64:128, :], k_sb[0:64, :])

                # V: cast to bf16, PE-transpose to token-major, append ones col
                vtmp = tmp_pool.tile([128, SC], BF16, tag="vtmp")
                nc.vector.tensor_copy(vtmp[64:128, :], kv_ps[64:128, :])
                for tb in range(4):
                    v1 = v1_pool.tile([128, 66], BF16, tag="v1")
                    V1_kb.append(v1)
                    vt_ps = acc_pool.tile([128, 64], BF16, tag="acc")
                    nc.tensor.transpose(
                        vt_ps[:], vtmp[64:128, tb * 128:(tb + 1) * 128],
                        ident[64:128, 64:128],
                    )
                    nc.vector.tensor_copy(v1[:, 0:64], vt_ps[:])
                    nc.vector.memset(v1[:, 64:65], 1.0)

            def attention(c0):
                nkb = 4 * (c0 + 1)
                a_pair = [ach_pool.tile([128, SC], BF16, tag="a",
                                        name=f"a_c{c0}p{i}")
                          for i in range(2)]
                A_ch.append(a_pair)
                oc_list = []
                for p in range(2):
                    op2 = oacc_pool.tile([65, 2, SC], F32, tag="oacc")
                    o0 = op2[:, 0, :]
                    o1 = op2[:, 1, :]
                    for kb in range(nkb):
                        kc = K_ch[kb // 4]
                        kcols = slice((kb % 4) * 128, (kb % 4 + 1) * 128)
                        s2 = s_pool.tile([128, 2, SC], F32, tag="s2")
                        nc.tensor.matmul(
                            s2[:, 0, :],
                            kc[0:64, kcols],
                            Q_ch[c0][0:64, p, :],
                            start=True, stop=True,
                        )
                        nc.tensor.matmul(
                            s2[:, 1, :],
                            kc[64:128, kcols],
                            Q_ch[c0][64:128, p, :],
                            start=True, stop=True,
                            tile_position=(64, 0),
                        )
                        # on diagonal blocks, only q-cols >= q0 are live
                        j = kb - (nkb - 4)
                        q0 = 128 * j if j >= 0 else 0
                        w = SC - q0
                        p2 = p_pool.tile([128, 2, SC], BF16, tag="p2")
                        nc.scalar.activation(p2[:, :, q0:], s2[:, :, q0:],
                                             EXP, scale=SCALE)
                        if j >= 0:
                            msk = mask_sb[:, 384: 896 - q0]
                            nc.vector.tensor_tensor(
                                p2[:, :, q0:], p2[:, :, q0:],
                                msk.unsqueeze(1).to_broadcast([128, 2, w]), MUL)
                        nc.tensor.matmul(
                            o0[:, q0:], V1_kb[kb][:, 0:65], p2[:, 0, q0:],
                            start=(kb == 0), stop=(kb == nkb - 1),
                        )
                        nc.tensor.matmul(
                            o1[:, q0:], V1_kb[kb][:, 0:65], p2[:, 1, q0:],
                            start=(kb == 0), stop=(kb == nkb - 1),
                        )
                    # evacuate PSUM accumulator to SBUF (frees the banks)
                    oc2 = oc_pool.tile([65, 2, SC], F32, tag="oc")
                    nc.vector.tensor_copy(oc2[:], op2[:])
                    oc_list.append(oc2)
                return a_pair, oc_list

            def divide_pair(a_tile, oc2):
                # reshape each [1, 512] sums row to [32, 16] so the
                # reciprocal runs on many DVE lanes instead of one
                rsum = r_pool.tile([64, 16], F32, tag="rsum")
                for hh in range(2):
                    nc.sync.dma_start(
                        rsum[32 * hh: 32 * hh + 32, :],
                        oc2[64:65, hh, :].rearrange("o (a n) -> o a n", a=32))
                rrecs = r_pool.tile([64, 16], F32R, tag="rrecs")
                with nc.allow_low_precision(reason="f32r-typed reciprocal output"):
                    nc.vector.reciprocal(rrecs[:], rsum[:])
                rrec = r_pool.tile([1, 2, SC], F32R, tag="rrec")
                for hh in range(2):
                    nc.sync.dma_start(
                        rrec[0:1, hh, :].rearrange("o (a n) -> o a n", a=32),
                        rrecs[32 * hh: 32 * hh + 32, :])
                for hh in range(2):
                    bc = acc_pool.tile([64, SC], F32, tag="acc")
                    nc.tensor.matmul(
                        bc[:], ones1[0:1, :], rrec[0:1, hh, :],
                        start=True, stop=True,
                    )
                    if hh == 0:
                        nc.vector.tensor_tensor(
                            a_tile[0:64, :], oc2[0:64, hh, :], bc[:], MUL)
                    else:
                        tb = r_pool.tile([64, SC], BF16, tag="tb")
                        nc.vector.tensor_tensor(
                            tb[:], oc2[0:64, hh, :], bc[:], MUL)
                        # move to partitions 64:128 (DMA crosses partitions)
                        nc.sync.dma_start(a_tile[64:128, :], tb[:])

            def out_proj(c0):
                ncols = slice(c0 * SC, (c0 + 1) * SC)
                for mb in range(8):
                    o_ps = acc_pool.tile([128, SC], F32, tag="acc")
                    for c in range(2):
                        nc.tensor.matmul(
                            o_ps[:],
                            wo_sb[:, c, mb * 128:(mb + 1) * 128],
                            A_ch[c0][c][:, :],
                            start=(c == 0), stop=(c == 1),
                        )
                    osb = o_pool.tile([128, SC], F32, tag="osb")
                    nc.vector.tensor_copy(osb[:], o_ps[:])
                    nc.sync.dma_start(out_v[:, mb, ncols], osb[:])

            proj(0)
            for c0 in range(NCH):
                a_pair, oc_list = attention(c0)
                if c0 + 1 < NCH:
                    proj(c0 + 1)       # fills the PE while division runs
                for p in range(2):
                    divide_pair(a_pair[p], oc_list[p])
                out_proj(c0)

    nc.compile()
    return nc


def shard_inputs(x, wq, wk, wv, wo, freqs_cos, freqs_sin):
    """Build the 8 per-core input maps (host-side layout prep)."""
    x = np.ascontiguousarray(np.asarray(x, dtype=np.float32))
    wq = np.asarray(wq, dtype=np.float32)
    wk = np.asarray(wk, dtype=np.float32)
    wv = np.asarray(wv, dtype=np.float32)
    wo = np.asarray(wo, dtype=np.float32)
    cos = np.asarray(freqs_cos, dtype=np.float32)   # [S, 32]
    sin = np.asarray(freqs_sin, dtype=np.float32)

    rope_cos = np.repeat(cos.T, 2, axis=0)          # [64, S]
    rope_sin = np.repeat(sin.T, 2, axis=0)
    rope_sin[0::2, :] *= -1.0                       # row 2i: -sin_i, 2i+1: +sin_i

    # causal keep-mask M[k, c] = 1 iff k <= c - 384; tiles slice cols
    kk = np.arange(128)[:, None]
    cc = np.arange(896)[None, :]
    mask01 = (kk <= cc - 384).astype(ml_dtypes.bfloat16)

    in_maps = []
    for core in range(N_CORES):
        b, r = divmod(core, TPG)
        xT = np.ascontiguousarray(x[b].T)                         # [DM, S]
        wq_s = wq[r * KFEAT:(r + 1) * KFEAT]                      # [256, DM]
        wk_s = wk[r * HD:(r + 1) * HD]                            # [64, DM]
        wv_s = wv[r * HD:(r + 1) * HD]
        wkvT = np.ascontiguousarray(
            np.concatenate([wk_s, wv_s], axis=0).T)               # [DM, 128]
        wqT = np.ascontiguousarray(wq_s.T)                        # [DM, 256]
        woT = np.ascontiguousarray(wo[:, r * KFEAT:(r + 1) * KFEAT].T)  # [256, DM]
        bf = ml_dtypes.bfloat16
        in_maps.append({
            "xT": xT.astype(bf),
            "wqT": wqT.astype(bf),
            "wkvT": wkvT.astype(bf),
            "woT": woT.astype(bf),
            "ropeCos": rope_cos,
            "ropeSin": rope_sin,
            "mask01": mask01,
        })
    return in_maps


def unshard(results):
    """Sum TP partials per batch and transpose back to [B, S, DM]."""
    out = np.empty((B, S, DM), dtype=np.float32)
    for b in range(B):
        acc = results[b * TPG]["out"].astype(np.float32).copy()
        for r in range(1, TPG):
            acc += results[b * TPG + r]["out"]
        out[b] = acc.T
    return out


def kernel(**inputs):
    global LAST_RESULTS, _NC_CACHE
    if _NC_CACHE is None:
        _NC_CACHE = build_nc()
    in_maps = shard_inputs(**inputs)
    LAST_RESULTS = run_bass_kernel_spmd(_NC_CACHE, in_maps, list(range(N_CORES)))
    return unshard(LAST_RESULTS.results)


# revision 30
# speedup vs baseline: 1.1501x; 1.0215x over previous
"""GQA attention (B=2, S=2048, DM=1024, H=16, KH=4, RoPE, causal) on 8 TRN2 cores.

Sharding: DP=2 over batch x TP=4 over heads. Core c handles batch c//4 and
q-heads [4r, 4r+4), kv-head r, where r = c % 4. Each core computes a partial
out^T = wo_shard @ attn_shard of shape [DM, S]; the host sums the 4 partials
per batch and transposes (gather/unshard).

Per-core kernel (single NEFF, SPMD):
  - Q/K computed feature-major ([feat, tok]) via transposed weight layouts
    prepared on host; V computed feature-major then PE-transposed to
    token-major with a ones-column appended (rowsum trick).
  - RoPE: adjacent-partition swap via stream_shuffle + elementwise muls with
    replicated cos / (+-)sin tables.
  - Causal softmax without max-subtraction (logits are bounded ~|2.5| for
    this distribution); exp on ACT engine over two PSUM banks at once
    (both heads of a pair); causal masking as a post-exp bf16 multiply on
    the 4 diagonal key-blocks only.
  - Scores matmuls run two heads concurrently in the two 64-row PE groups
    (K duplicated to partitions 64..127).
  - All dense matmuls in float32r (full PE rate at free-dim 512); AV in bf16.
"""

import numpy as np
import ml_dtypes

import concourse.bass as bass
import concourse.mybir as mybir
import concourse.tile as tile
from concourse import bacc
from concourse.bass_utils import run_bass_kernel_spmd
from concourse.masks import make_identity

F32 = mybir.dt.float32
F32R = mybir.dt.float32r
BF16 = mybir.dt.bfloat16

B, S, DM, H, KH, HD = 2, 2048, 1024, 16, 4, 64
N_CORES = 8
TPG = 4                 # tensor-parallel group size
QH = H // TPG           # q-heads per core
KFEAT = QH * HD         # 256 q-features per core
SC = 512                # token chunk
NCH = S // SC           # 4
KB = 128                # key block
NKB = S // KB           # 16
SCALE = 1.0 / np.sqrt(HD)
XOR1 = [i ^ 1 for i in range(32)]

LAST_RESULTS = None     # BassKernelResults of the most recent run (for test.py)
_NC_CACHE = None


def build_nc():
    nc = bacc.Bacc("TRN2", target_bir_lowering=False, debug=False,
                   num_devices=N_CORES)

    xT = nc.declare_dram_parameter("xT", [DM, S], BF16, isOutput=False)
    wqT = nc.declare_dram_parameter("wqT", [DM, KFEAT], BF16, isOutput=False)
    wkvT = nc.declare_dram_parameter("wkvT", [DM, 128], BF16, isOutput=False)
    woT = nc.declare_dram_parameter("woT", [KFEAT, DM], BF16, isOutput=False)
    ropeCos = nc.declare_dram_parameter("ropeCos", [64, S], F32, isOutput=False)
    ropeSin = nc.declare_dram_parameter("ropeSin", [64, S], F32, isOutput=False)
    mask01 = nc.declare_dram_parameter("mask01", [128, 896], BF16, isOutput=False)
    out = nc.declare_dram_parameter("out", [DM, S], F32, isOutput=True)

    xT_v = xT.rearrange("(kb p) n -> p kb n", p=128)        # [128, 8, S]
    wqT_v = wqT.rearrange("(kb p) m -> p kb m", p=128)      # [128, 8, 256]
    wkvT_v = wkvT.rearrange("(kb p) m -> p kb m", p=128)    # [128, 8, 128]
    woT_v = woT.rearrange("(c p) n -> p c n", p=128)        # [128, 2, 1024]
    out_v = out.rearrange("(mb p) n -> p mb n", p=128)      # [128, 8, S]

    EXP = mybir.ActivationFunctionType.Exp
    MUL = bass.mybir.AluOpType.mult
    ADD = bass.mybir.AluOpType.add

    with tile.TileContext(nc) as tc:
        with (
            tc.tile_pool(name="consts", bufs=1) as consts,
            tc.tile_pool(name="kch", bufs=NCH) as kch_pool,
            tc.tile_pool(name="qch", bufs=NCH) as qch_pool,
            tc.tile_pool(name="ach", bufs=2 * NCH) as ach_pool,
            tc.tile_pool(name="v1p", bufs=NKB) as v1_pool,
            tc.tile_pool(name="xch", bufs=2) as xch_pool,
            tc.tile_pool(name="tmp", bufs=2) as tmp_pool,
            tc.tile_pool(name="pp", bufs=5) as p_pool,
            tc.tile_pool(name="rp", bufs=2) as r_pool,
            tc.tile_pool(name="op", bufs=3) as o_pool,
            tc.tile_pool(name="ocp", bufs=6) as oc_pool,
            tc.tile_pool(name="acc", bufs=2, space="PSUM") as acc_pool,
            tc.tile_pool(name="oac", bufs=1, space="PSUM") as oacc_pool,
            tc.tile_pool(name="sme", bufs=2, space="PSUM") as s_pool,
        ):
            # ---- constants ----
            wq_sb = consts.tile([128, 8, KFEAT], BF16, tag="wq")
            wkv_sb = consts.tile([128, 8, 128], BF16, tag="wkv")
            wo_sb = consts.tile([128, 2, DM], BF16, tag="wo")
            cos_sb = consts.tile([128, S], F32, tag="cos")
            sin_sb = consts.tile([128, S], F32, tag="sin")
            mask_sb = consts.tile([128, 896], BF16, tag="mask")
            ident = consts.tile([128, 128], BF16, tag="ident")

            nc.sync.dma_start(wq_sb[:], wqT_v)
            nc.sync.dma_start(wkv_sb[:], wkvT_v)
            nc.sync.dma_start(wo_sb[:], woT_v)
            nc.sync.dma_start(cos_sb[0:64, :], ropeCos[:])
            nc.sync.dma_start(cos_sb[64:128, :], ropeCos[:])
            nc.sync.dma_start(sin_sb[0:64, :], ropeSin[:])
            nc.sync.dma_start(sin_sb[64:128, :], ropeSin[:])
            nc.sync.dma_start(mask_sb[:], mask01[:])
            make_identity(nc, ident[:])
            ones1f = consts.tile([1, 64], F32, tag="ones1f")
            ones1 = consts.tile([1, 64], F32R, tag="ones1")
            nc.vector.memset(ones1f[:], 1.0)
            nc.vector.tensor_copy(ones1[:], ones1f[:])

            K_ch = []       # per-chunk K, feature-major, duplicated rows
            Q_ch = []       # per-chunk Q, feature-major, [128, 2, SC]
            A_ch = []       # per-chunk attn output, feature-major
            V1_kb = []      # per key-block token-major [V | 1]

            def proj(c0):
                cols = slice(c0 * SC, (c0 + 1) * SC)
                x_sb = xch_pool.tile([128, 8, SC], BF16, tag="x")
                nc.sync.dma_start(x_sb[:], xT_v[:, :, cols])

                q_sb = qch_pool.tile([128, 2, SC], BF16, tag="q")
                k_sb = kch_pool.tile([128, SC], BF16, tag="k")
                Q_ch.append(q_sb)
                K_ch.append(k_sb)

                # Q projection + RoPE, two 128-feature tiles (2 heads each)
                for m in range(2):
                    q_ps = acc_pool.tile([128, SC], F32, tag="acc")
                    for kb in range(8):
                        nc.tensor.matmul(
                            q_ps[:],
                            wq_sb[:, kb, m * 128:(m + 1) * 128],
                            x_sb[:, kb, :],
                            start=(kb == 0), stop=(kb == 7),
                        )
                    qsw = tmp_pool.tile([128, SC], F32, tag="qsw")
                    t1 = tmp_pool.tile([128, SC], F32, tag="t1")
                    t2 = tmp_pool.tile([128, SC], F32, tag="t2")
                    nc.vector.stream_shuffle(qsw[:], q_ps[:], XOR1)
                    nc.vector.tensor_tensor(t1[:], q_ps[:], cos_sb[:, cols], MUL)
                    nc.vector.tensor_tensor(t2[:], qsw[:], sin_sb[:, cols], MUL)
                    nc.vector.tensor_tensor(q_sb[:, m, :], t1[:], t2[:], ADD)

                # K (rows 0:64) and V (rows 64:128) projection
                kv_ps = acc_pool.tile([128, SC], F32, tag="acc")
                for kb in range(8):
                    nc.tensor.matmul(
                        kv_ps[:],
                        wkv_sb[:, kb, :],
                        x_sb[:, kb, :],
                        start=(kb == 0), stop=(kb == 7),
                    )
                # K RoPE
                ksw = tmp_pool.tile([64, SC], F32, tag="ksw")
                t1k = tmp_pool.tile([64, SC], F32, tag="t1k")
                t2k = tmp_pool.tile([64, SC], F32, tag="t2k")
                nc.vector.stream_shuffle(ksw[:], kv_ps[0:64, :], XOR1)
                nc.vector.tensor_tensor(t1k[:], kv_ps[0:64, :], cos_sb[0:64, cols], MUL)
                nc.vector.tensor_tensor(t2k[:], ksw[:], sin_sb[0:64, cols], MUL)
                nc.vector.tensor_tensor(k_sb[0:64, :], t1k[:], t2k[:], ADD)
                # duplicate K into partitions 64:128 (second PE row group)
                nc.sync.dma_start(k_sb[64:128, :], k_sb[0:64, :])

                # V: cast to bf16, PE-transpose to token-major, append ones col
                vtmp = tmp_pool.tile([128, SC], BF16, tag="vtmp")
                nc.vector.tensor_copy(vtmp[64:128, :], kv_ps[64:128, :])
                for tb in range(4):
                    v1 = v1_pool.tile([128, 66], BF16, tag="v1")
                    V1_kb.append(v1)
                    vt_ps = acc_pool.tile([128, 64], BF16, tag="acc")
                    nc.tensor.transpose(
                        vt_ps[:], vtmp[64:128, tb * 128:(tb + 1) * 128],
                        ident[64:128, 64:128],
                    )
                    nc.vector.tensor_copy(v1[:, 0:64], vt_ps[:])
                    nc.vector.memset(v1[:, 64:65], 1.0)

            def attention(c0):
                nkb = 4 * (c0 + 1)
                a_pair = [ach_pool.tile([128, SC], BF16, tag="a",
                                        name=f"a_c{c0}p{i}")
                          for i in range(2)]
                A_ch.append(a_pair)
                oc_list = []
                for p in range(2):
                    op2 = oacc_pool.tile([65, 2, SC], F32, tag="oacc")
                    o0 = op2[:, 0, :]
                    o1 = op2[:, 1, :]
                    for kb in range(nkb):
                        kc = K_ch[kb // 4]
                        kcols = slice((kb % 4) * 128, (kb % 4 + 1) * 128)
                        s2 = s_pool.tile([128, 2, SC], F32, tag="s2")
                        nc.tensor.matmul(
                            s2[:, 0, :],
                            kc[0:64, kcols],
                            Q_ch[c0][0:64, p, :],
                            start=True, stop=True,
                        )
                        nc.tensor.matmul(
                            s2[:, 1, :],
                            kc[64:128, kcols],
                            Q_ch[c0][64:128, p, :],
                            start=True, stop=True,
                            tile_position=(64, 0),
                        )
                        # on diagonal blocks, only q-cols >= q0 are live
                        j = kb - (nkb - 4)
                        q0 = 128 * j if j >= 0 else 0
                        w = SC - q0
                        p2 = p_pool.tile([128, 2, SC], BF16, tag="p2")
                        nc.scalar.activation(p2[:, :, q0:], s2[:, :, q0:],
                                             EXP, scale=SCALE)
                        if j >= 0:
                            msk = mask_sb[:, 384: 896 - q0]
                            nc.vector.tensor_tensor(
                                p2[:, :, q0:], p2[:, :, q0:],
                                msk.unsqueeze(1).to_broadcast([128, 2, w]), MUL)
                        nc.tensor.matmul(
                            o0[:, q0:], V1_kb[kb][:, 0:65], p2[:, 0, q0:],
                            start=(kb == 0), stop=(kb == nkb - 1),
                        )
                        nc.tensor.matmul(
                            o1[:, q0:], V1_kb[kb][:, 0:65], p2[:, 1, q0:],
                            start=(kb == 0), stop=(kb == nkb - 1),
                        )
                    # evacuate PSUM accumulator to SBUF (frees the banks)
                    oc2 = oc_pool.tile([65, 2, SC], F32, tag="oc")
                    nc.vector.tensor_copy(oc2[:], op2[:])
                    oc_list.append(oc2)
                return a_pair, oc_list

            def divide_pair(a_tile, oc2):
                # reshape each [1, 512] sums row to [32, 16] so the
                # reciprocal runs on many DVE lanes instead of one
                rsum = r_pool.tile([64, 16], F32, tag="rsum")
                for hh in range(2):
                    nc.sync.dma_start(
                        rsum[32 * hh: 32 * hh + 32, :],
                        oc2[64:65, hh, :].rearrange("o (a n) -> o a n", a=32))
                rrecs = r_pool.tile([64, 16], F32, tag="rrecs")
                nc.vector.reciprocal(rrecs[:], rsum[:])
                rrec = r_pool.tile([1, 2, SC], F32, tag="rrec")
                for hh in range(2):
                    nc.sync.dma_start(
                        rrec[0:1, hh, :].rearrange("o (a n) -> o a n", a=32),
                        rrecs[32 * hh: 32 * hh + 32, :])
                for hh in range(2):
                    bc = r_pool.tile([64, SC], F32, tag="bc")
                    nc.gpsimd.partition_broadcast(bc[:], rrec[0:1, hh, :])
                    if hh == 0:
                        nc.vector.tensor_tensor(
                            a_tile[0:64, :], oc2[0:64, hh, :], bc[:], MUL)
                    else:
                        tb = r_pool.tile([64, SC], BF16, tag="tb")
                        nc.vector.tensor_tensor(
                            tb[:], oc2[0:64, hh, :], bc[:], MUL)
                        # move to partitions 64:128 (DMA crosses partitions)
                        nc.sync.dma_start(a_tile[64:128, :], tb[:])

            def out_proj(c0):
                ncols = slice(c0 * SC, (c0 + 1) * SC)
                for mb in range(8):
                    o_ps = acc_pool.tile([128, SC], F32, tag="acc")
                    for c in range(2):
                        nc.tensor.matmul(
                            o_ps[:],
                            wo_sb[:, c, mb * 128:(mb + 1) * 128],
                            A_ch[c0][c][:, :],
                            start=(c == 0), stop=(c == 1),
                        )
                    osb = o_pool.tile([128, SC], F32, tag="osb")
                    nc.vector.tensor_copy(osb[:], o_ps[:])
                    nc.sync.dma_start(out_v[:, mb, ncols], osb[:])

            proj(0)
            for c0 in range(NCH):
                a_pair, oc_list = attention(c0)
                if c0 + 1 < NCH:
                    proj(c0 + 1)       # fills the PE while division runs
                for p in range(2):
                    divide_pair(a_pair[p], oc_list[p])
                out_proj(c0)

    nc.compile()
    return nc


def shard_inputs(x, wq, wk, wv, wo, freqs_cos, freqs_sin):
    """Build the 8 per-core input maps (host-side layout prep)."""
    x = np.ascontiguousarray(np.asarray(x, dtype=np.float32))
    wq = np.asarray(wq, dtype=np.float32)
    wk = np.asarray(wk, dtype=np.float32)
    wv = np.asarray(wv, dtype=np.float32)
    wo = np.asarray(wo, dtype=np.float32)
    cos = np.asarray(freqs_cos, dtype=np.float32)   # [S, 32]
    sin = np.asarray(freqs_sin, dtype=np.float32)

    rope_cos = np.repeat(cos.T, 2, axis=0)          # [64, S]
    rope_sin = np.repeat(sin.T, 2, axis=0)
    rope_sin[0::2, :] *= -1.0                       # row 2i: -sin_i, 2i+1: +sin_i

    # causal keep-mask M[k, c] = 1 iff k <= c - 384; tiles slice cols
    kk = np.arange(128)[:, None]
    cc = np.arange(896)[None, :]
    mask01 = (kk <= cc - 384).astype(ml_dtypes.bfloat16)

    in_maps = []
    for core in range(N_CORES):
        b, r = divmod(core, TPG)
        xT = np.ascontiguousarray(x[b].T)                         # [DM, S]
        wq_s = wq[r * KFEAT:(r + 1) * KFEAT]                      # [256, DM]
        wk_s = wk[r * HD:(r + 1) * HD]                            # [64, DM]
        wv_s = wv[r * HD:(r + 1) * HD]
        wkvT = np.ascontiguousarray(
            np.concatenate([wk_s, wv_s], axis=0).T)               # [DM, 128]
        wqT = np.ascontiguousarray(wq_s.T)                        # [DM, 256]
        woT = np.ascontiguousarray(wo[:, r * KFEAT:(r + 1) * KFEAT].T)  # [256, DM]
        bf = ml_dtypes.bfloat16
        in_maps.append({
            "xT": xT.astype(bf),
            "wqT": wqT.astype(bf),
            "wkvT": wkvT.astype(bf),
            "woT": woT.astype(bf),
            "ropeCos": rope_cos,
            "ropeSin": rope_sin,
            "mask01": mask01,
        })
    return in_maps


def unshard(results):
    """Sum TP partials per batch and transpose back to [B, S, DM]."""
    out = np.empty((B, S, DM), dtype=np.float32)
    for b in range(B):
        acc = results[b * TPG]["out"].astype(np.float32).copy()
        for r in range(1, TPG):
            acc += results[b * TPG + r]["out"]
        out[b] = acc.T
    return out


def kernel(**inputs):
    global LAST_RESULTS, _NC_CACHE
    if _NC_CACHE is None:
        _NC_CACHE = build_nc()
    in_maps = shard_inputs(**inputs)
    LAST_RESULTS = run_bass_kernel_spmd(_NC_CACHE, in_maps, list(range(N_CORES)))
    return unshard(LAST_RESULTS.results)


# revision 31
# speedup vs baseline: 1.1620x; 1.0103x over previous
"""GQA attention (B=2, S=2048, DM=1024, H=16, KH=4, RoPE, causal) on 8 TRN2 cores.

Sharding: DP=2 over batch x TP=4 over heads. Core c handles batch c//4 and
q-heads [4r, 4r+4), kv-head r, where r = c % 4. Each core computes a partial
out^T = wo_shard @ attn_shard of shape [DM, S]; the host sums the 4 partials
per batch and transposes (gather/unshard).

Per-core kernel (single NEFF, SPMD):
  - Q/K computed feature-major ([feat, tok]) via transposed weight layouts
    prepared on host; V computed feature-major then PE-transposed to
    token-major with a ones-column appended (rowsum trick).
  - RoPE: adjacent-partition swap via stream_shuffle + elementwise muls with
    replicated cos / (+-)sin tables.
  - Causal softmax without max-subtraction (logits are bounded ~|2.5| for
    this distribution); exp on ACT engine over two PSUM banks at once
    (both heads of a pair); causal masking as a post-exp bf16 multiply on
    the 4 diagonal key-blocks only.
  - Scores matmuls run two heads concurrently in the two 64-row PE groups
    (K duplicated to partitions 64..127).
  - All dense matmuls in float32r (full PE rate at free-dim 512); AV in bf16.
"""

import numpy as np
import ml_dtypes

import concourse.bass as bass
import concourse.mybir as mybir
import concourse.tile as tile
from concourse import bacc
from concourse.bass_utils import run_bass_kernel_spmd
from concourse.masks import make_identity

F32 = mybir.dt.float32
F32R = mybir.dt.float32r
BF16 = mybir.dt.bfloat16

B, S, DM, H, KH, HD = 2, 2048, 1024, 16, 4, 64
N_CORES = 8
TPG = 4                 # tensor-parallel group size
QH = H // TPG           # q-heads per core
KFEAT = QH * HD         # 256 q-features per core
SC = 512                # token chunk
NCH = S // SC           # 4
KB = 128                # key block
NKB = S // KB           # 16
SCALE = 1.0 / np.sqrt(HD)
XOR1 = [i ^ 1 for i in range(32)]

LAST_RESULTS = None     # BassKernelResults of the most recent run (for test.py)
_NC_CACHE = None


def build_nc():
    nc = bacc.Bacc("TRN2", target_bir_lowering=False, debug=False,
                   num_devices=N_CORES)

    xT = nc.declare_dram_parameter("xT", [DM, S], BF16, isOutput=False)
    wqT = nc.declare_dram_parameter("wqT", [DM, KFEAT], BF16, isOutput=False)
    wkvT = nc.declare_dram_parameter("wkvT", [DM, 128], BF16, isOutput=False)
    woT = nc.declare_dram_parameter("woT", [KFEAT, DM], BF16, isOutput=False)
    ropeCos = nc.declare_dram_parameter("ropeCos", [64, S], F32, isOutput=False)
    ropeSin = nc.declare_dram_parameter("ropeSin", [64, S], F32, isOutput=False)
    mask01 = nc.declare_dram_parameter("mask01", [128, 896], BF16, isOutput=False)
    out = nc.declare_dram_parameter("out", [DM, S], F32, isOutput=True)

    xT_v = xT.rearrange("(kb p) n -> p kb n", p=128)        # [128, 8, S]
    wqT_v = wqT.rearrange("(kb p) m -> p kb m", p=128)      # [128, 8, 256]
    wkvT_v = wkvT.rearrange("(kb p) m -> p kb m", p=128)    # [128, 8, 128]
    woT_v = woT.rearrange("(c p) n -> p c n", p=128)        # [128, 2, 1024]
    out_v = out.rearrange("(mb p) n -> p mb n", p=128)      # [128, 8, S]

    EXP = mybir.ActivationFunctionType.Exp
    MUL = bass.mybir.AluOpType.mult
    ADD = bass.mybir.AluOpType.add

    with tile.TileContext(nc) as tc:
        with (
            tc.tile_pool(name="consts", bufs=1) as consts,
            tc.tile_pool(name="kch", bufs=NCH) as kch_pool,
            tc.tile_pool(name="qch", bufs=NCH) as qch_pool,
            tc.tile_pool(name="ach", bufs=2 * NCH) as ach_pool,
            tc.tile_pool(name="v1p", bufs=NKB) as v1_pool,
            tc.tile_pool(name="xch", bufs=2) as xch_pool,
            tc.tile_pool(name="tmp", bufs=2) as tmp_pool,
            tc.tile_pool(name="pp", bufs=5) as p_pool,
            tc.tile_pool(name="rp", bufs=2) as r_pool,
            tc.tile_pool(name="op", bufs=3) as o_pool,
            tc.tile_pool(name="ocp", bufs=6) as oc_pool,
            tc.tile_pool(name="acc", bufs=2, space="PSUM") as acc_pool,
            tc.tile_pool(name="oac", bufs=1, space="PSUM") as oacc_pool,
            tc.tile_pool(name="sme", bufs=2, space="PSUM") as s_pool,
        ):
            # ---- constants ----
            wq_sb = consts.tile([128, 8, KFEAT], BF16, tag="wq")
            wkv_sb = consts.tile([128, 8, 128], BF16, tag="wkv")
            wo_sb = consts.tile([128, 2, DM], BF16, tag="wo")
            cos_sb = consts.tile([128, S], F32, tag="cos")
            sin_sb = consts.tile([128, S], F32, tag="sin")
            mask_sb = consts.tile([128, 896], BF16, tag="mask")
            ident = consts.tile([128, 128], BF16, tag="ident")

            nc.sync.dma_start(wq_sb[:], wqT_v)
            nc.sync.dma_start(wkv_sb[:], wkvT_v)
            nc.sync.dma_start(wo_sb[:], woT_v)
            nc.sync.dma_start(cos_sb[0:64, :], ropeCos[:])
            nc.sync.dma_start(cos_sb[64:128, :], ropeCos[:])
            nc.sync.dma_start(sin_sb[0:64, :], ropeSin[:])
            nc.sync.dma_start(sin_sb[64:128, :], ropeSin[:])
            nc.sync.dma_start(mask_sb[:], mask01[:])
            make_identity(nc, ident[:])
            ones1f = consts.tile([1, 64], F32, tag="ones1f")
            ones1 = consts.tile([1, 64], F32R, tag="ones1")
            nc.vector.memset(ones1f[:], 1.0)
            nc.vector.tensor_copy(ones1[:], ones1f[:])

            K_ch = []       # per-chunk K, feature-major, duplicated rows
            Q_ch = []       # per-chunk Q, feature-major, [128, 2, SC]
            A_ch = []       # per-chunk attn output, feature-major
            V1_kb = []      # per key-block token-major [V | 1]

            def proj_load(c0):
                cols = slice(c0 * SC, (c0 + 1) * SC)
                x_sb = xch_pool.tile([128, 8, SC], BF16, tag="x")
                nc.sync.dma_start(x_sb[:], xT_v[:, :, cols])
                return x_sb

            def proj_q(c0, x_sb):
                cols = slice(c0 * SC, (c0 + 1) * SC)
                q_sb = qch_pool.tile([128, 2, SC], BF16, tag="q")
                Q_ch.append(q_sb)

                # Q projection + RoPE, two 128-feature tiles (2 heads each)
                for m in range(2):
                    q_ps = acc_pool.tile([128, SC], F32, tag="acc")
                    for kb in range(8):
                        nc.tensor.matmul(
                            q_ps[:],
                            wq_sb[:, kb, m * 128:(m + 1) * 128],
                            x_sb[:, kb, :],
                            start=(kb == 0), stop=(kb == 7),
                        )
                    qsw = tmp_pool.tile([128, SC], F32, tag="qsw")
                    t1 = tmp_pool.tile([128, SC], F32, tag="t1")
                    t2 = tmp_pool.tile([128, SC], F32, tag="t2")
                    nc.vector.stream_shuffle(qsw[:], q_ps[:], XOR1)
                    nc.vector.tensor_tensor(t1[:], q_ps[:], cos_sb[:, cols], MUL)
                    nc.vector.tensor_tensor(t2[:], qsw[:], sin_sb[:, cols], MUL)
                    nc.vector.tensor_tensor(q_sb[:, m, :], t1[:], t2[:], ADD)

            def proj_kv(c0, x_sb):
                cols = slice(c0 * SC, (c0 + 1) * SC)
                k_sb = kch_pool.tile([128, SC], BF16, tag="k")
                K_ch.append(k_sb)
                # K (rows 0:64) and V (rows 64:128) projection
                kv_ps = acc_pool.tile([128, SC], F32, tag="acc")
                for kb in range(8):
                    nc.tensor.matmul(
                        kv_ps[:],
                        wkv_sb[:, kb, :],
                        x_sb[:, kb, :],
                        start=(kb == 0), stop=(kb == 7),
                    )
                # K RoPE
                ksw = tmp_pool.tile([64, SC], F32, tag="ksw")
                t1k = tmp_pool.tile([64, SC], F32, tag="t1k")
                t2k = tmp_pool.tile([64, SC], F32, tag="t2k")
                nc.vector.stream_shuffle(ksw[:], kv_ps[0:64, :], XOR1)
                nc.vector.tensor_tensor(t1k[:], kv_ps[0:64, :], cos_sb[0:64, cols], MUL)
                nc.vector.tensor_tensor(t2k[:], ksw[:], sin_sb[0:64, cols], MUL)
                nc.vector.tensor_tensor(k_sb[0:64, :], t1k[:], t2k[:], ADD)
                # duplicate K into partitions 64:128 (second PE row group)
                nc.sync.dma_start(k_sb[64:128, :], k_sb[0:64, :])

                # V: cast to bf16, PE-transpose to token-major, append ones col
                vtmp = tmp_pool.tile([128, SC], BF16, tag="vtmp")
                nc.vector.tensor_copy(vtmp[64:128, :], kv_ps[64:128, :])
                for tb in range(4):
                    v1 = v1_pool.tile([128, 66], BF16, tag="v1")
                    V1_kb.append(v1)
                    vt_ps = acc_pool.tile([128, 64], BF16, tag="acc")
                    nc.tensor.transpose(
                        vt_ps[:], vtmp[64:128, tb * 128:(tb + 1) * 128],
                        ident[64:128, 64:128],
                    )
                    nc.vector.tensor_copy(v1[:, 0:64], vt_ps[:])
                    nc.vector.memset(v1[:, 64:65], 1.0)

            def attention_pair(c0, p, a_pair):
                nkb = 4 * (c0 + 1)
                if True:
                    op2 = oacc_pool.tile([65, 2, SC], F32, tag="oacc")
                    o0 = op2[:, 0, :]
                    o1 = op2[:, 1, :]
                    for kb in range(nkb):
                        kc = K_ch[kb // 4]
                        kcols = slice((kb % 4) * 128, (kb % 4 + 1) * 128)
                        s2 = s_pool.tile([128, 2, SC], F32, tag="s2")
                        nc.tensor.matmul(
                            s2[:, 0, :],
                            kc[0:64, kcols],
                            Q_ch[c0][0:64, p, :],
                            start=True, stop=True,
                        )
                        nc.tensor.matmul(
                            s2[:, 1, :],
                            kc[64:128, kcols],
                            Q_ch[c0][64:128, p, :],
                            start=True, stop=True,
                            tile_position=(64, 0),
                        )
                        # on diagonal blocks, only q-cols >= q0 are live
                        j = kb - (nkb - 4)
                        q0 = 128 * j if j >= 0 else 0
                        w = SC - q0
                        p2 = p_pool.tile([128, 2, SC], BF16, tag="p2")
                        nc.scalar.activation(p2[:, :, q0:], s2[:, :, q0:],
                                             EXP, scale=SCALE)
                        if j >= 0:
                            msk = mask_sb[:, 384: 896 - q0]
                            nc.vector.tensor_tensor(
                                p2[:, :, q0:], p2[:, :, q0:],
                                msk.unsqueeze(1).to_broadcast([128, 2, w]), MUL)
                        nc.tensor.matmul(
                            o0[:, q0:], V1_kb[kb][:, 0:65], p2[:, 0, q0:],
                            start=(kb == 0), stop=(kb == nkb - 1),
                        )
                        nc.tensor.matmul(
                            o1[:, q0:], V1_kb[kb][:, 0:65], p2[:, 1, q0:],
                            start=(kb == 0), stop=(kb == nkb - 1),
                        )
                    # evacuate PSUM accumulator to SBUF (frees the banks)
                    oc2 = oc_pool.tile([65, 2, SC], F32, tag="oc")
                    nc.vector.tensor_copy(oc2[:], op2[:])
                return oc2

            def divide_pair(a_tile, oc2):
                # reshape each [1, 512] sums row to [32, 16] so the
                # reciprocal runs on many DVE lanes instead of one
                rsum = r_pool.tile([64, 16], F32, tag="rsum")
                for hh in range(2):
                    nc.sync.dma_start(
                        rsum[32 * hh: 32 * hh + 32, :],
                        oc2[64:65, hh, :].rearrange("o (a n) -> o a n", a=32))
                rrecs = r_pool.tile([64, 16], F32, tag="rrecs")
                nc.vector.reciprocal(rrecs[:], rsum[:])
                rrec = r_pool.tile([1, 2, SC], F32, tag="rrec")
                for hh in range(2):
                    nc.sync.dma_start(
                        rrec[0:1, hh, :].rearrange("o (a n) -> o a n", a=32),
                        rrecs[32 * hh: 32 * hh + 32, :])
                for hh in range(2):
                    bc = r_pool.tile([64, SC], F32, tag="bc")
                    nc.gpsimd.partition_broadcast(bc[:], rrec[0:1, hh, :])
                    if hh == 0:
                        nc.vector.tensor_tensor(
                            a_tile[0:64, :], oc2[0:64, hh, :], bc[:], MUL)
                    else:
                        tb = r_pool.tile([64, SC], BF16, tag="tb")
                        nc.vector.tensor_tensor(
                            tb[:], oc2[0:64, hh, :], bc[:], MUL)
                        # move to partitions 64:128 (DMA crosses partitions)
                        nc.sync.dma_start(a_tile[64:128, :], tb[:])

            def out_proj(c0):
                ncols = slice(c0 * SC, (c0 + 1) * SC)
                for mb in range(8):
                    o_ps = acc_pool.tile([128, SC], F32, tag="acc")
                    for c in range(2):
                        nc.tensor.matmul(
                            o_ps[:],
                            wo_sb[:, c, mb * 128:(mb + 1) * 128],
                            A_ch[c0][c][:, :],
                            start=(c == 0), stop=(c == 1),
                        )
                    osb = o_pool.tile([128, SC], F32, tag="osb")
                    nc.vector.tensor_copy(osb[:], o_ps[:])
                    nc.sync.dma_start(out_v[:, mb, ncols], osb[:])

            x0 = proj_load(0)
            proj_q(0, x0)
            proj_kv(0, x0)
            xn = None
            for c0 in range(NCH):
                a_pair = [ach_pool.tile([128, SC], BF16, tag="a",
                                        name=f"a_c{c0}p{i}")
                          for i in range(2)]
                A_ch.append(a_pair)
                if c0 + 1 < NCH:
                    xn = proj_load(c0 + 1)
                oc0 = attention_pair(c0, 0, a_pair)
                if c0 + 1 < NCH:
                    proj_q(c0 + 1, xn)  # fills PE bubbles during attention
                divide_pair(a_pair[0], oc0)
                oc1 = attention_pair(c0, 1, a_pair)
                if c0 + 1 < NCH:
                    proj_kv(c0 + 1, xn)
                divide_pair(a_pair[1], oc1)
                out_proj(c0)

    nc.compile()
    return nc


def shard_inputs(x, wq, wk, wv, wo, freqs_cos, freqs_sin):
    """Build the 8 per-core input maps (host-side layout prep)."""
    x = np.ascontiguousarray(np.asarray(x, dtype=np.float32))
    wq = np.asarray(wq, dtype=np.float32)
    wk = np.asarray(wk, dtype=np.float32)
    wv = np.asarray(wv, dtype=np.float32)
    wo = np.asarray(wo, dtype=np.float32)
    cos = np.asarray(freqs_cos, dtype=np.float32)   # [S, 32]
    sin = np.asarray(freqs_sin, dtype=np.float32)

    rope_cos = np.repeat(cos.T, 2, axis=0)          # [64, S]
    rope_sin = np.repeat(sin.T, 2, axis=0)
    rope_sin[0::2, :] *= -1.0                       # row 2i: -sin_i, 2i+1: +sin_i

    # causal keep-mask M[k, c] = 1 iff k <= c - 384; tiles slice cols
    kk = np.arange(128)[:, None]
    cc = np.arange(896)[None, :]
    mask01 = (kk <= cc - 384).astype(ml_dtypes.bfloat16)

    in_maps = []
    for core in range(N_CORES):
        b, r = divmod(core, TPG)
        xT = np.ascontiguousarray(x[b].T)                         # [DM, S]
        wq_s = wq[r * KFEAT:(r + 1) * KFEAT]                      # [256, DM]
        wk_s = wk[r * HD:(r + 1) * HD]                            # [64, DM]
        wv_s = wv[r * HD:(r + 1) * HD]
        wkvT = np.ascontiguousarray(
            np.concatenate([wk_s, wv_s], axis=0).T)               # [DM, 128]
        wqT = np.ascontiguousarray(wq_s.T)                        # [DM, 256]
        woT = np.ascontiguousarray(wo[:, r * KFEAT:(r + 1) * KFEAT].T)  # [256, DM]
        bf = ml_dtypes.bfloat16
        in_maps.append({
            "xT": xT.astype(bf),
            "wqT": wqT.astype(bf),
            "wkvT": wkvT.astype(bf),
            "woT": woT.astype(bf),
            "ropeCos": rope_cos,
            "ropeSin": rope_sin,
            "mask01": mask01,
        })
    return in_maps


def unshard(results):
    """Sum TP partials per batch and transpose back to [B, S, DM]."""
    out = np.empty((B, S, DM), dtype=np.float32)
    for b in range(B):
        acc = results[b * TPG]["out"].astype(np.float32).copy()
        for r in range(1, TPG):
            acc += results[b * TPG + r]["out"]
        out[b] = acc.T
    return out


def kernel(**inputs):
    global LAST_RESULTS, _NC_CACHE
    if _NC_CACHE is None:
        _NC_CACHE = build_nc()
    in_maps = shard_inputs(**inputs)
    LAST_RESULTS = run_bass_kernel_spmd(_NC_CACHE, in_maps, list(range(N_CORES)))
    return unshard(LAST_RESULTS.results)
